# revision 9
# baseline (speedup 1.0000x reference)
"""GRU kernel for Trainium2, 8 NeuronCores, data-parallel over batch.

Problem: B=64, S=1024, I=H=1024 GRU (bias_ih only).
  gi = x @ W_ih.T + b_ih            (big parallel GEMM)
  per step: gh = h @ W_hh.T; gates; h' = (1-z)*n + z*h   (serial, S steps)

Sharding: batch 64 -> 8 per core; weights replicated. All matmul operands
bf16 (fp32 matmul is 4x slower on PE; bf16 numerics ~3e-3 rel-l2).

Layouts (per core, BL=8 local batch): hidden dim on partitions, batch in
free dim ("transposed"), so the serial recurrence needs no on-chip
transposes and gate math uses all 128 partitions.
  hT  [128, k*8+b]  = h[b, k*128+p]           (k = H-chunk 0..7)
  ghT psum [128, m*8+b] for m-tile m (0..23 over 3H)
  gi DRAM [128, t*192 + m*8 + b]  (step slices contiguous [128,192])
x is host-pre-transposed to [ic, 128, S, BL] bf16; y is emitted in the
native [128, t*64+k*8+b] layout and un-transposed on host.
"""

import os
import sys

import numpy as np
import ml_dtypes

for _p in ("/opt/trn_rl_repo",):
    if _p not in sys.path:
        sys.path.insert(0, _p)

import concourse.bass as bass
import concourse.bacc as bacc
import concourse.mybir as mybir
import concourse.bass_utils as _bu
from concourse import tile
from concourse.bass_utils import run_bass_kernel_spmd

B, S, I, H = 64, 1024, 1024, 1024
NC = 8
BL = B // NC            # 8
G3 = 3 * H              # 3072
MT = G3 // 128          # 24 m-tiles over 3H
KT = H // 128           # 8 k-chunks over H
ICT = I // 128          # 8 i-chunks over I
TCG = 64                # gemm chunk (steps)
TCR = 64                # recurrence chunk (steps)
F32 = mybir.dt.float32
BF16 = mybir.dt.bfloat16
SIG = mybir.ActivationFunctionType.Sigmoid
TANH = mybir.ActivationFunctionType.Tanh

# Walrus's LDWEIGHTS optimization (fast weight load) is disabled by the
# default driver flags; the recurrence is LDW-bound so it matters here.
if int(os.environ.get("GRU_LDWOPT", "0")):
    _orig_run_command = _bu.run_command

    def _run_command_ldwopt(argv, **kw):
        argv = [
            "--enable-ldw-opt=true" if a == "--enable-ldw-opt=false" else a
            for a in argv
        ]
        return _orig_run_command(argv, **kw)

    _bu.run_command = _run_command_ldwopt


def build():
    nc = bacc.Bacc("TRN2", target_bir_lowering=False, debug=False)
    xT = nc.declare_dram_parameter("xT", [ICT, 128, S, BL], BF16, isOutput=False)
    h0T = nc.declare_dram_parameter("h0T", [128, KT * BL], F32, isOutput=False)
    WihT = nc.declare_dram_parameter("WihT", [ICT, 128, G3], BF16, isOutput=False)
    WhhT = nc.declare_dram_parameter("WhhT", [KT, 128, G3], BF16, isOutput=False)
    bih = nc.declare_dram_parameter("bih", [128, MT], F32, isOutput=False)
    y_raw = nc.declare_dram_parameter("y_raw", [128, S * KT * BL], F32, isOutput=True)
    hx_raw = nc.declare_dram_parameter("hx_raw", [128, KT * BL], F32, isOutput=True)
    gi_dram = nc.dram_tensor("gi_buf", [128, S * MT * BL], F32)

    with tile.TileContext(nc) as tc:
        # ---------------- phase 1: gi = x @ W_ih.T + b_ih -> DRAM ----------
        with (
            tc.tile_pool(name="w1", bufs=1) as wpool,
            tc.tile_pool(name="p1", bufs=4, space="PSUM") as ppool,
            tc.tile_pool(name="s1", bufs=2) as spool,
        ):
            wih_sb = wpool.tile([128, ICT * G3], BF16)
            for ic in range(ICT):
                nc.sync.dma_start(wih_sb[:, ic * G3 : (ic + 1) * G3], WihT[ic])
            b_sb = wpool.tile([128, MT], F32)
            nc.sync.dma_start(b_sb[:], bih[:])

            N1 = TCG * BL  # 512 moving columns per chunk
            for ci in range(S // TCG):
                xt_sb = spool.tile([128, ICT, N1], BF16, tag="xt")
                for ic in range(ICT):
                    nc.sync.dma_start(
                        xt_sb[:, ic, :],
                        xT[ic][:, ci * TCG : (ci + 1) * TCG, :],
                    )
                stage = spool.tile([128, TCG, MT * BL], F32, tag="gistage")
                for m in range(MT):
                    ps = ppool.tile([128, TCG, BL], F32, tag="ps1")
                    for k in range(ICT):
                        nc.tensor.matmul(
                            ps[:],
                            wih_sb[:, k * G3 + m * 128 : k * G3 + (m + 1) * 128],
                            xt_sb[:, k, :],
                            start=(k == 0),
                            stop=(k == ICT - 1),
                        )
                    # scatter into per-step layout, fused + bias
                    nc.vector.tensor_scalar_add(
                        stage[:, :, m * BL : (m + 1) * BL],
                        ps[:],
                        b_sb[:, m : m + 1],
                    )
                nc.sync.dma_start(
                    gi_dram[:, ci * (TCG * MT * BL) : (ci + 1) * (TCG * MT * BL)],
                    stage[:],
                )

        # ---------------- phase 2: recurrence ------------------------------
        with (
            tc.tile_pool(name="w2", bufs=1) as wpool2,
            tc.tile_pool(name="pers", bufs=1) as pers,
            tc.tile_pool(name="p2", bufs=2, space="PSUM") as ppool2,
            tc.tile_pool(name="s2", bufs=2) as spool2,
            tc.tile_pool(name="tmp", bufs=2) as tpool,
        ):
            whh_sb = wpool2.tile([128, KT * G3], BF16)
            for k in range(KT):
                nc.sync.dma_start(whh_sb[:, k * G3 : (k + 1) * G3], WhhT[k])

            hTf = pers.tile([128, KT * BL], F32)
            hTb = pers.tile([128, KT * BL], BF16)
            nc.sync.dma_start(hTf[:], h0T[:])
            nc.vector.tensor_copy(hTb[:], hTf[:])

            NG = MT * BL            # 192 gate columns per step
            N2 = TCR * NG           # gi chunk columns
            NY = KT * BL            # 64

            with tc.For_i(0, S // TCR, 1) as ci:
                gi_sb = spool2.tile([128, N2], F32, tag="gi")
                # quarter-DMAs so early steps only wait on the first slice
                q = N2 // 4
                for j in range(4):
                    nc.sync.dma_start(
                        gi_sb[:, j * q : (j + 1) * q],
                        gi_dram[:, bass.ds(ci * N2 + j * q, q)],
                    )
                y_stage = spool2.tile([128, TCR * NY], F32, tag="y")

                for t in range(TCR):
                    gis = gi_sb[:, t * NG : (t + 1) * NG]
                    h_prev = hTf[:] if t == 0 else y_stage[:, (t - 1) * NY : t * NY]
                    ps_rza = ppool2.tile([128, 128], F32, tag="psrza")
                    ps_rzb = ppool2.tile([128, 128], F32, tag="psrzb")
                    ps_n0 = ppool2.tile([128, NY // 2], F32, tag="psn0")
                    ps_n1 = ppool2.tile([128, NY // 2], F32, tag="psn1")

                    def mms(m_lo, m_hi, pout, poff, kr):
                        for m in range(m_lo, m_hi):
                            for k in kr:
                                nc.tensor.matmul(
                                    pout[:, (m - poff) * BL : (m - poff + 1) * BL],
                                    whh_sb[
                                        :, k * G3 + m * 128 : k * G3 + (m + 1) * 128
                                    ],
                                    hTb[:, k * BL : (k + 1) * BL],
                                    start=(k == kr[0]),
                                    stop=(k == kr[-1]),
                                )

                    # rz-part: k-halves as independent accumulation groups so
                    # the step's first matmuls only need the first half of h
                    mms(0, 16, ps_rza, 0, [0, 1, 2, 3])
                    mms(0, 16, ps_rzb, 0, [4, 5, 6, 7])
                    # n-part in two m-halves (H-chunks 0-3, then 4-7)
                    mms(16, 20, ps_n0, 16, [0, 1, 2, 3, 4, 5, 6, 7])
                    mms(20, 24, ps_n1, 20, [0, 1, 2, 3, 4, 5, 6, 7])

                    # --- overlaps the n-part matmuls ---
                    s_rz = tpool.tile([128, 128], F32, tag="srz")
                    nc.vector.tensor_add(s_rz[:], ps_rza[:], gis[:, 0:128])
                    nc.vector.tensor_add(s_rz[:], s_rz[:], ps_rzb[:])
                    rz = tpool.tile([128, 128], F32, tag="rz")
                    nc.scalar.activation(rz[:], s_rz[:], SIG)
                    zh = tpool.tile([128, NY], F32, tag="zh")
                    nc.vector.tensor_mul(zh[:], rz[:, NY:128], h_prev)
                    omz = tpool.tile([128, NY], F32, tag="omz")
                    nc.vector.tensor_scalar(
                        omz[:], rz[:, NY:128], -1.0, 1.0,
                        mybir.AluOpType.mult, mybir.AluOpType.add,
                    )
                    # --- tails per n-half; half 0 overlaps half 1's matmuls,
                    # half 1 overlaps the next step's first k-half ---
                    NH = NY // 2
                    for j, ps_n in ((0, ps_n0), (1, ps_n1)):
                        sl = slice(j * NH, (j + 1) * NH)
                        u = tpool.tile([128, NH], F32, tag=f"u{j}")
                        nc.vector.tensor_mul(u[:], rz[:, j * NH : (j + 1) * NH], ps_n[:])
                        v = tpool.tile([128, NH], F32, tag=f"v{j}")
                        nc.vector.tensor_add(
                            v[:], u[:], gis[:, 128 + j * NH : 128 + (j + 1) * NH]
                        )
                        nst = tpool.tile([128, NH], F32, tag=f"nst{j}")
                        nc.scalar.activation(nst[:], v[:], TANH)
                        h1 = tpool.tile([128, NH], F32, tag=f"h1{j}")
                        nc.vector.tensor_mul(h1[:], nst[:], omz[:, sl])
                        # bf16 h half for next step's matmuls (output-cast add)
                        nc.vector.tensor_add(hTb[:, sl], h1[:], zh[:, sl])
                        # fp32 h (= y_t) off the critical path
                        nc.vector.tensor_add(
                            y_stage[:, t * NY + j * NH : t * NY + (j + 1) * NH],
                            h1[:],
                            zh[:, sl],
                        )

                nc.vector.tensor_copy(hTf[:], y_stage[:, (TCR - 1) * NY :])
                nc.sync.dma_start(
                    y_raw[:, bass.ds(ci * (TCR * NY), TCR * NY)], y_stage[:]
                )
            nc.sync.dma_start(hx_raw[:], hTf[:])
    nc.compile()
    return nc


_CACHE = {}


def _prep_inputs(x, h0, W_ih, W_hh, b_ih):
    bf = ml_dtypes.bfloat16
    x = np.ascontiguousarray(np.asarray(x, dtype=np.float32))
    h0 = np.asarray(h0, dtype=np.float32)
    W_ih = np.asarray(W_ih, dtype=np.float32)
    W_hh = np.asarray(W_hh, dtype=np.float32)
    b_ih = np.asarray(b_ih, dtype=np.float32)

    # shared weights
    wihT = np.ascontiguousarray(W_ih.T).reshape(ICT, 128, G3).astype(bf)
    whhT = np.ascontiguousarray(W_hh.T).reshape(KT, 128, G3).astype(bf)
    b_arr = np.ascontiguousarray(b_ih.reshape(MT, 128).T)

    in_maps = []
    for c in range(NC):
        x_c = x[c * BL : (c + 1) * BL]                      # [BL, S, I]
        xT_c = np.ascontiguousarray(x_c.transpose(2, 1, 0)) # [I, S, BL]
        xT_c = xT_c.reshape(ICT, 128, S, BL).astype(bf)
        h0_c = h0[c * BL : (c + 1) * BL]                    # [BL, H]
        h0T_c = np.ascontiguousarray(
            h0_c.reshape(BL, KT, 128).transpose(2, 1, 0).reshape(128, KT * BL)
        )
        in_maps.append(
            {"xT": xT_c, "h0T": h0T_c, "WihT": wihT, "WhhT": whhT, "bih": b_arr}
        )
    return in_maps


def _postprocess(results):
    y_full = np.empty((B, S, H), dtype=np.float32)
    hx_full = np.empty((B, H), dtype=np.float32)
    for c in range(NC):
        y_raw = results[c]["y_raw"]        # [128, S*KT*BL]
        hx_raw = results[c]["hx_raw"]      # [128, KT*BL]
        y = y_raw.reshape(128, S, KT, BL).transpose(3, 1, 2, 0).reshape(BL, S, H)
        y_full[c * BL : (c + 1) * BL] = y
        hx = hx_raw.reshape(128, KT, BL).transpose(2, 1, 0).reshape(BL, H)
        hx_full[c * BL : (c + 1) * BL] = hx
    return y_full, hx_full


def kernel(x, h0, W_ih, W_hh, b_ih):
    if "nc" not in _CACHE:
        _CACHE["nc"] = build()
    nc = _CACHE["nc"]
    in_maps = _prep_inputs(x, h0, W_ih, W_hh, b_ih)
    trace = bool(int(os.environ.get("GRU_TRACE", "0")))
    res = run_bass_kernel_spmd(nc, in_maps, list(range(NC)), trace=trace)
    _CACHE["last_result"] = res
    return _postprocess(res.results)


if __name__ == "__main__":
    rng = np.random.default_rng(0)
    sc = 1.0 / np.sqrt(H)
    inputs = {
        "x": rng.standard_normal((B, S, I), dtype=np.float32),
        "h0": np.zeros((B, H), dtype=np.float32),
        "W_ih": (rng.standard_normal((G3, I), dtype=np.float32) * sc),
        "W_hh": (rng.standard_normal((G3, H), dtype=np.float32) * sc),
        "b_ih": (rng.standard_normal(G3, dtype=np.float32) * sc),
    }
    y, hx = kernel(**inputs)
    print("ok", y.shape, hx.shape, float(np.abs(y).max()))


# revision 11
# speedup vs baseline: 1.2018x; 1.2018x over previous
"""GRU kernel for Trainium2, 8 NeuronCores, data-parallel over batch.

Problem: B=64, S=1024, I=H=1024 GRU (bias_ih only).
  gi = x @ W_ih.T + b_ih            (big parallel GEMM)
  per step: gh = h @ W_hh.T; gates; h' = (1-z)*n + z*h   (serial, S steps)

Sharding: batch 64 -> 8 per core; weights replicated. All matmul operands
bf16 (fp32 matmul is 4x slower on PE; bf16 numerics ~3e-3 rel-l2).

Layouts (per core, BL=8 local batch): hidden dim on partitions, batch in
free dim ("transposed"), so the serial recurrence needs no on-chip
transposes and gate math uses all 128 partitions.
  hT  [128, k*8+b]  = h[b, k*128+p]           (k = H-chunk 0..7)
  ghT psum [128, m*8+b] for m-tile m (0..23 over 3H)
  gi DRAM [128, t*192 + m*8 + b]  (step slices contiguous [128,192])
x is host-pre-transposed to [ic, 128, S, BL] bf16; y is emitted in the
native [128, t*64+k*8+b] layout and un-transposed on host.
"""

import os
import sys

import numpy as np
import ml_dtypes

for _p in ("/opt/trn_rl_repo",):
    if _p not in sys.path:
        sys.path.insert(0, _p)

import concourse.bass as bass
import concourse.bacc as bacc
import concourse.mybir as mybir
import concourse.bass_utils as _bu
from concourse import tile
from concourse.tile import add_dep_helper
from concourse.bass_utils import run_bass_kernel_spmd

B, S, I, H = 64, 1024, 1024, 1024
NC = 8
BL = B // NC            # 8
G3 = 3 * H              # 3072
MT = G3 // 128          # 24 m-tiles over 3H
KT = H // 128           # 8 k-chunks over H
ICT = I // 128          # 8 i-chunks over I
TCG = 64                # gemm chunk (steps)
TCR = 64                # recurrence chunk (steps)
F32 = mybir.dt.float32
BF16 = mybir.dt.bfloat16
SIG = mybir.ActivationFunctionType.Sigmoid
TANH = mybir.ActivationFunctionType.Tanh

# Walrus's LDWEIGHTS optimization (fast weight load) is disabled by the
# default driver flags; the recurrence is LDW-bound so it matters here.
if int(os.environ.get("GRU_LDWOPT", "0")):
    _orig_run_command = _bu.run_command

    def _run_command_ldwopt(argv, **kw):
        argv = [
            "--enable-ldw-opt=true" if a == "--enable-ldw-opt=false" else a
            for a in argv
        ]
        return _orig_run_command(argv, **kw)

    _bu.run_command = _run_command_ldwopt


def build():
    nc = bacc.Bacc("TRN2", target_bir_lowering=False, debug=False)
    xT = nc.declare_dram_parameter("xT", [ICT, 128, S, BL], BF16, isOutput=False)
    h0T = nc.declare_dram_parameter("h0T", [128, KT * BL], F32, isOutput=False)
    WihT = nc.declare_dram_parameter("WihT", [ICT, 128, G3], BF16, isOutput=False)
    WhhT = nc.declare_dram_parameter("WhhT", [KT, 128, G3], BF16, isOutput=False)
    bih = nc.declare_dram_parameter("bih", [128, MT], F32, isOutput=False)
    y_raw = nc.declare_dram_parameter("y_raw", [128, S * KT * BL], F32, isOutput=True)
    hx_raw = nc.declare_dram_parameter("hx_raw", [128, KT * BL], F32, isOutput=True)
    gi_dram = nc.dram_tensor("gi_buf", [128, S * MT * BL], F32)

    with tile.TileContext(nc) as tc:
        # ---------------- phase 1: gi = x @ W_ih.T + b_ih -> DRAM ----------
        with (
            tc.tile_pool(name="w1", bufs=1) as wpool,
            tc.tile_pool(name="p1", bufs=4, space="PSUM") as ppool,
            tc.tile_pool(name="s1", bufs=2) as spool,
        ):
            wih_sb = wpool.tile([128, ICT * G3], BF16)
            for ic in range(ICT):
                nc.sync.dma_start(wih_sb[:, ic * G3 : (ic + 1) * G3], WihT[ic])
            b_sb = wpool.tile([128, MT], F32)
            nc.sync.dma_start(b_sb[:], bih[:])

            N1 = TCG * BL  # 512 moving columns per chunk
            for ci in range(S // TCG):
                xt_sb = spool.tile([128, ICT, N1], BF16, tag="xt")
                for ic in range(ICT):
                    nc.sync.dma_start(
                        xt_sb[:, ic, :],
                        xT[ic][:, ci * TCG : (ci + 1) * TCG, :],
                    )
                stage = spool.tile([128, TCG, MT * BL], F32, tag="gistage")
                for m in range(MT):
                    ps = ppool.tile([128, TCG, BL], F32, tag="ps1")
                    for k in range(ICT):
                        nc.tensor.matmul(
                            ps[:],
                            wih_sb[:, k * G3 + m * 128 : k * G3 + (m + 1) * 128],
                            xt_sb[:, k, :],
                            start=(k == 0),
                            stop=(k == ICT - 1),
                        )
                    # scatter into per-step layout, fused + bias
                    nc.vector.tensor_scalar_add(
                        stage[:, :, m * BL : (m + 1) * BL],
                        ps[:],
                        b_sb[:, m : m + 1],
                    )
                nc.sync.dma_start(
                    gi_dram[:, ci * (TCG * MT * BL) : (ci + 1) * (TCG * MT * BL)],
                    stage[:],
                )

        # ---------------- phase 2: recurrence ------------------------------
        with (
            tc.tile_pool(name="w2", bufs=1) as wpool2,
            tc.tile_pool(name="pers", bufs=1) as pers,
            tc.tile_pool(name="p2", bufs=2, space="PSUM") as ppool2,
            tc.tile_pool(name="s2", bufs=2) as spool2,
            tc.tile_pool(name="tmp", bufs=2) as tpool,
        ):
            whh_sb = wpool2.tile([128, KT * G3], BF16)
            for k in range(KT):
                nc.sync.dma_start(whh_sb[:, k * G3 : (k + 1) * G3], WhhT[k])

            hTf = pers.tile([128, KT * BL], F32)
            hTb = pers.tile([128, KT * BL], BF16)
            nc.sync.dma_start(hTf[:], h0T[:])
            nc.vector.tensor_copy(hTb[:], hTf[:])

            NG = MT * BL            # 192 gate columns per step
            N2 = TCR * NG           # gi chunk columns
            NY = KT * BL            # 64

            with tc.For_i(0, S // TCR, 1) as ci:
                gi_sb = spool2.tile([128, N2], F32, tag="gi")
                # quarter-DMAs so early steps only wait on the first slice
                q = N2 // 4
                for j in range(4):
                    nc.sync.dma_start(
                        gi_sb[:, j * q : (j + 1) * q],
                        gi_dram[:, bass.ds(ci * N2 + j * q, q)],
                    )
                y_stage = spool2.tile([128, TCR * NY], F32, tag="y")

                for t in range(TCR):
                    gis = gi_sb[:, t * NG : (t + 1) * NG]
                    h_prev = hTf[:] if t == 0 else y_stage[:, (t - 1) * NY : t * NY]
                    ps_rza = ppool2.tile([128, 128], F32, tag="psrza")
                    ps_rzb = ppool2.tile([128, 128], F32, tag="psrzb")
                    ps_n0 = ppool2.tile([128, NY // 2], F32, tag="psn0")
                    ps_n1 = ppool2.tile([128, NY // 2], F32, tag="psn1")

                    def mms(m_lo, m_hi, pout, poff, kr):
                        for m in range(m_lo, m_hi):
                            for k in kr:
                                nc.tensor.matmul(
                                    pout[:, (m - poff) * BL : (m - poff + 1) * BL],
                                    whh_sb[
                                        :, k * G3 + m * 128 : k * G3 + (m + 1) * 128
                                    ],
                                    hTb[:, k * BL : (k + 1) * BL],
                                    start=(k == kr[0]),
                                    stop=(k == kr[-1]),
                                )

                    # rz-part: k-halves as independent accumulation groups so
                    # the step's first matmuls only need the first half of h
                    mms(0, 16, ps_rza, 0, [0, 1, 2, 3])
                    mms(0, 16, ps_rzb, 0, [4, 5, 6, 7])
                    # n-part in two m-halves (H-chunks 0-3, then 4-7)
                    mms(16, 20, ps_n0, 16, [0, 1, 2, 3, 4, 5, 6, 7])
                    mms(20, 24, ps_n1, 20, [0, 1, 2, 3, 4, 5, 6, 7])

                    # --- overlaps the n-part matmuls ---
                    s_rz = tpool.tile([128, 128], F32, tag="srz")
                    nc.vector.tensor_add(s_rz[:], ps_rza[:], gis[:, 0:128])
                    nc.vector.tensor_add(s_rz[:], s_rz[:], ps_rzb[:])
                    rz = tpool.tile([128, 128], F32, tag="rz")
                    nc.scalar.activation(rz[:], s_rz[:], SIG)
                    zh = tpool.tile([128, NY], F32, tag="zh")
                    nc.vector.tensor_mul(zh[:], rz[:, NY:128], h_prev)
                    omz = tpool.tile([128, NY], F32, tag="omz")
                    nc.vector.tensor_scalar(
                        omz[:], rz[:, NY:128], -1.0, 1.0,
                        mybir.AluOpType.mult, mybir.AluOpType.add,
                    )
                    # --- tails per n-half; half 0 overlaps half 1's matmuls,
                    # half 1 overlaps the next step's first k-half ---
                    NH = NY // 2
                    prev_y = None
                    for j, ps_n in ((0, ps_n0), (1, ps_n1)):
                        sl = slice(j * NH, (j + 1) * NH)
                        u = tpool.tile([128, NH], F32, tag=f"u{j}")
                        iu = nc.vector.tensor_mul(
                            u[:], rz[:, j * NH : (j + 1) * NH], ps_n[:]
                        )
                        if prev_y is not None:
                            # keep DVE program order: half-1 chain strictly
                            # after half-0's, so half-0 finishes early
                            add_dep_helper(
                                iu.ins, prev_y.ins, sync=False,
                                reason="pipeline n-halves",
                            )
                        v = tpool.tile([128, NH], F32, tag=f"v{j}")
                        nc.vector.tensor_add(
                            v[:], u[:], gis[:, 128 + j * NH : 128 + (j + 1) * NH]
                        )
                        nst = tpool.tile([128, NH], F32, tag=f"nst{j}")
                        nc.scalar.activation(nst[:], v[:], TANH)
                        h1 = tpool.tile([128, NH], F32, tag=f"h1{j}")
                        nc.vector.tensor_mul(h1[:], nst[:], omz[:, sl])
                        # bf16 h half for next step's matmuls (output-cast add)
                        ib = nc.vector.tensor_add(hTb[:, sl], h1[:], zh[:, sl])
                        # fp32 h (= y_t) off the critical path
                        iy = nc.vector.tensor_add(
                            y_stage[:, t * NY + j * NH : t * NY + (j + 1) * NH],
                            h1[:],
                            zh[:, sl],
                        )
                        add_dep_helper(
                            iy.ins, ib.ins, sync=False, reason="y after hTb"
                        )
                        prev_y = iy

                nc.vector.tensor_copy(hTf[:], y_stage[:, (TCR - 1) * NY :])
                nc.sync.dma_start(
                    y_raw[:, bass.ds(ci * (TCR * NY), TCR * NY)], y_stage[:]
                )
            nc.sync.dma_start(hx_raw[:], hTf[:])
    nc.compile()
    return nc


_CACHE = {}


def _prep_inputs(x, h0, W_ih, W_hh, b_ih):
    bf = ml_dtypes.bfloat16
    x = np.ascontiguousarray(np.asarray(x, dtype=np.float32))
    h0 = np.asarray(h0, dtype=np.float32)
    W_ih = np.asarray(W_ih, dtype=np.float32)
    W_hh = np.asarray(W_hh, dtype=np.float32)
    b_ih = np.asarray(b_ih, dtype=np.float32)

    # shared weights
    wihT = np.ascontiguousarray(W_ih.T).reshape(ICT, 128, G3).astype(bf)
    whhT = np.ascontiguousarray(W_hh.T).reshape(KT, 128, G3).astype(bf)
    b_arr = np.ascontiguousarray(b_ih.reshape(MT, 128).T)

    in_maps = []
    for c in range(NC):
        x_c = x[c * BL : (c + 1) * BL]                      # [BL, S, I]
        xT_c = np.ascontiguousarray(x_c.transpose(2, 1, 0)) # [I, S, BL]
        xT_c = xT_c.reshape(ICT, 128, S, BL).astype(bf)
        h0_c = h0[c * BL : (c + 1) * BL]                    # [BL, H]
        h0T_c = np.ascontiguousarray(
            h0_c.reshape(BL, KT, 128).transpose(2, 1, 0).reshape(128, KT * BL)
        )
        in_maps.append(
            {"xT": xT_c, "h0T": h0T_c, "WihT": wihT, "WhhT": whhT, "bih": b_arr}
        )
    return in_maps


def _postprocess(results):
    y_full = np.empty((B, S, H), dtype=np.float32)
    hx_full = np.empty((B, H), dtype=np.float32)
    for c in range(NC):
        y_raw = results[c]["y_raw"]        # [128, S*KT*BL]
        hx_raw = results[c]["hx_raw"]      # [128, KT*BL]
        y = y_raw.reshape(128, S, KT, BL).transpose(3, 1, 2, 0).reshape(BL, S, H)
        y_full[c * BL : (c + 1) * BL] = y
        hx = hx_raw.reshape(128, KT, BL).transpose(2, 1, 0).reshape(BL, H)
        hx_full[c * BL : (c + 1) * BL] = hx
    return y_full, hx_full


def kernel(x, h0, W_ih, W_hh, b_ih):
    if "nc" not in _CACHE:
        _CACHE["nc"] = build()
    nc = _CACHE["nc"]
    in_maps = _prep_inputs(x, h0, W_ih, W_hh, b_ih)
    trace = bool(int(os.environ.get("GRU_TRACE", "0")))
    res = run_bass_kernel_spmd(nc, in_maps, list(range(NC)), trace=trace)
    _CACHE["last_result"] = res
    return _postprocess(res.results)


if __name__ == "__main__":
    rng = np.random.default_rng(0)
    sc = 1.0 / np.sqrt(H)
    inputs = {
        "x": rng.standard_normal((B, S, I), dtype=np.float32),
        "h0": np.zeros((B, H), dtype=np.float32),
        "W_ih": (rng.standard_normal((G3, I), dtype=np.float32) * sc),
        "W_hh": (rng.standard_normal((G3, H), dtype=np.float32) * sc),
        "b_ih": (rng.standard_normal(G3, dtype=np.float32) * sc),
    }
    y, hx = kernel(**inputs)
    print("ok", y.shape, hx.shape, float(np.abs(y).max()))


# revision 15
# speedup vs baseline: 1.2821x; 1.0668x over previous
"""GRU kernel for Trainium2, 8 NeuronCores, data-parallel over batch.

Problem: B=64, S=1024, I=H=1024 GRU (bias_ih only).
  gi = x @ W_ih.T + b_ih            (big parallel GEMM)
  per step: gh = h @ W_hh.T; gates; h' = (1-z)*n + z*h   (serial, S steps)

Sharding: batch 64 -> 8 per core; weights replicated. All matmul operands
bf16 (fp32 matmul is 4x slower on PE; bf16 numerics ~3e-3 rel-l2).

Layouts (per core, BL=8 local batch): hidden dim on partitions, batch in
free dim ("transposed"), so the serial recurrence needs no on-chip
transposes and gate math uses all 128 partitions.
  hT  [128, k*8+b]  = h[b, k*128+p]           (k = H-chunk 0..7)
  ghT psum [128, m*8+b] for m-tile m (0..23 over 3H)
  gi DRAM [128, t*192 + m*8 + b]  (step slices contiguous [128,192])
x is host-pre-transposed to [ic, 128, S, BL] bf16; y is emitted in the
native [128, t*64+k*8+b] layout and un-transposed on host.
"""

import os
import sys

import numpy as np
import ml_dtypes

for _p in ("/opt/trn_rl_repo",):
    if _p not in sys.path:
        sys.path.insert(0, _p)

import concourse.bass as bass
import concourse.bacc as bacc
import concourse.mybir as mybir
import concourse.bass_utils as _bu
from concourse import tile
from concourse.tile import add_dep_helper
from concourse.bass_utils import run_bass_kernel_spmd

B, S, I, H = 64, 1024, 1024, 1024
NC = 8
BL = B // NC            # 8
G3 = 3 * H              # 3072
MT = G3 // 128          # 24 m-tiles over 3H
KT = H // 128           # 8 k-chunks over H
ICT = I // 128          # 8 i-chunks over I
TCG = 64                # gemm chunk (steps)
TCR = 64                # recurrence chunk (steps)
F32 = mybir.dt.float32
BF16 = mybir.dt.bfloat16
SIG = mybir.ActivationFunctionType.Sigmoid
TANH = mybir.ActivationFunctionType.Tanh

# Walrus's LDWEIGHTS optimization (fast weight load) is disabled by the
# default driver flags; the recurrence is LDW-bound so it matters here.
if int(os.environ.get("GRU_LDWOPT", "0")):
    _orig_run_command = _bu.run_command

    def _run_command_ldwopt(argv, **kw):
        argv = [
            "--enable-ldw-opt=true" if a == "--enable-ldw-opt=false" else a
            for a in argv
        ]
        return _orig_run_command(argv, **kw)

    _bu.run_command = _run_command_ldwopt


def build():
    nc = bacc.Bacc("TRN2", target_bir_lowering=False, debug=False)
    xT = nc.declare_dram_parameter("xT", [ICT, 128, S, BL], BF16, isOutput=False)
    h0T = nc.declare_dram_parameter("h0T", [128, KT * BL], F32, isOutput=False)
    WihT = nc.declare_dram_parameter("WihT", [ICT, 128, G3], BF16, isOutput=False)
    WhhT = nc.declare_dram_parameter("WhhT", [KT, 128, G3], BF16, isOutput=False)
    bih = nc.declare_dram_parameter("bih", [128, MT], F32, isOutput=False)
    y_raw = nc.declare_dram_parameter("y_raw", [128, S * KT * BL], F32, isOutput=True)
    hx_raw = nc.declare_dram_parameter("hx_raw", [128, KT * BL], F32, isOutput=True)
    gi_dram = nc.dram_tensor("gi_buf", [128, S * MT * BL], F32)

    with tile.TileContext(nc) as tc:
        # ---------------- phase 1: gi = x @ W_ih.T + b_ih -> DRAM ----------
        with (
            tc.tile_pool(name="w1", bufs=1) as wpool,
            tc.tile_pool(name="p1", bufs=4, space="PSUM") as ppool,
            tc.tile_pool(name="s1", bufs=2) as spool,
        ):
            wih_sb = wpool.tile([128, ICT * G3], BF16)
            for ic in range(ICT):
                nc.sync.dma_start(wih_sb[:, ic * G3 : (ic + 1) * G3], WihT[ic])
            b_sb = wpool.tile([128, MT], F32)
            nc.sync.dma_start(b_sb[:], bih[:])

            N1 = TCG * BL  # 512 moving columns per chunk
            for ci in range(S // TCG):
                xt_sb = spool.tile([128, ICT, N1], BF16, tag="xt")
                for ic in range(ICT):
                    nc.sync.dma_start(
                        xt_sb[:, ic, :],
                        xT[ic][:, ci * TCG : (ci + 1) * TCG, :],
                    )
                stage = spool.tile([128, TCG, MT * BL], F32, tag="gistage")
                for m in range(MT):
                    ps = ppool.tile([128, TCG, BL], F32, tag="ps1")
                    for k in range(ICT):
                        nc.tensor.matmul(
                            ps[:],
                            wih_sb[:, k * G3 + m * 128 : k * G3 + (m + 1) * 128],
                            xt_sb[:, k, :],
                            start=(k == 0),
                            stop=(k == ICT - 1),
                        )
                    # scatter into per-step layout, fused + bias
                    nc.vector.tensor_scalar_add(
                        stage[:, :, m * BL : (m + 1) * BL],
                        ps[:],
                        b_sb[:, m : m + 1],
                    )
                nc.sync.dma_start(
                    gi_dram[:, ci * (TCG * MT * BL) : (ci + 1) * (TCG * MT * BL)],
                    stage[:],
                )

        # ---------------- phase 2: recurrence ------------------------------
        with (
            tc.tile_pool(name="w2", bufs=1) as wpool2,
            tc.tile_pool(name="pers", bufs=1) as pers,
            tc.tile_pool(name="p2", bufs=2, space="PSUM") as ppool2,
            tc.tile_pool(name="s2", bufs=2) as spool2,
            tc.tile_pool(name="tmp", bufs=2) as tpool,
        ):
            whh_sb = wpool2.tile([128, KT * G3], BF16)
            for k in range(KT):
                nc.sync.dma_start(whh_sb[:, k * G3 : (k + 1) * G3], WhhT[k])

            hTf = pers.tile([128, KT * BL], F32)
            hTb = pers.tile([128, KT * BL], BF16)
            nc.sync.dma_start(hTf[:], h0T[:])
            nc.vector.tensor_copy(hTb[:], hTf[:])

            NG = MT * BL            # 192 gate columns per step
            N2 = TCR * NG           # gi chunk columns
            NY = KT * BL            # 64

            with tc.For_i(0, S // TCR, 1) as ci:
                gi_sb = spool2.tile([128, N2], F32, tag="gi")
                # quarter-DMAs so early steps only wait on the first slice
                q = N2 // 4
                for j in range(4):
                    nc.sync.dma_start(
                        gi_sb[:, j * q : (j + 1) * q],
                        gi_dram[:, bass.ds(ci * N2 + j * q, q)],
                    )
                y_stage = spool2.tile([128, TCR * NY], F32, tag="y")

                prev_step_y = None
                prev_step_tanh = None
                for t in range(TCR):
                    gis = gi_sb[:, t * NG : (t + 1) * NG]
                    h_prev = hTf[:] if t == 0 else y_stage[:, (t - 1) * NY : t * NY]
                    ps_rza = ppool2.tile([128, 128], F32, tag="psrza")
                    ps_rzb = ppool2.tile([128, 128], F32, tag="psrzb")
                    ps_n0 = ppool2.tile([128, NY // 2], F32, tag="psn0")
                    ps_n1 = ppool2.tile([128, NY // 2], F32, tag="psn1")

                    def mms(m_lo, m_hi, pout, poff, kr):
                        for m in range(m_lo, m_hi):
                            for k in kr:
                                nc.tensor.matmul(
                                    pout[:, (m - poff) * BL : (m - poff + 1) * BL],
                                    whh_sb[
                                        :, k * G3 + m * 128 : k * G3 + (m + 1) * 128
                                    ],
                                    hTb[:, k * BL : (k + 1) * BL],
                                    start=(k == kr[0]),
                                    stop=(k == kr[-1]),
                                )

                    # rz-part: k-halves as independent accumulation groups so
                    # the step's first matmuls only need the first half of h
                    mms(0, 16, ps_rza, 0, [0, 1, 2, 3])
                    mms(0, 16, ps_rzb, 0, [4, 5, 6, 7])
                    # n-part in two m-halves (H-chunks 0-3, then 4-7)
                    mms(16, 20, ps_n0, 16, [0, 1, 2, 3, 4, 5, 6, 7])
                    mms(20, 24, ps_n1, 20, [0, 1, 2, 3, 4, 5, 6, 7])

                    # --- overlaps the n-part matmuls ---
                    s_rz = tpool.tile([128, 128], F32, tag="srz")
                    ia = nc.vector.tensor_add(s_rz[:], ps_rza[:], gis[:, 0:128])
                    if prev_step_y is not None:
                        # DVE program order: this step's work strictly after
                        # the previous step's tail
                        add_dep_helper(
                            ia.ins, prev_step_y.ins, sync=False,
                            reason="step chain dve",
                        )
                    nc.vector.tensor_add(s_rz[:], s_rz[:], ps_rzb[:])
                    rz = tpool.tile([128, 128], F32, tag="rz")
                    isg = nc.scalar.activation(rz[:], s_rz[:], SIG)
                    if prev_step_tanh is not None:
                        add_dep_helper(
                            isg.ins, prev_step_tanh.ins, sync=False,
                            reason="step chain act",
                        )
                    zh = tpool.tile([128, NY], F32, tag="zh")
                    nc.vector.tensor_mul(zh[:], rz[:, NY:128], h_prev)
                    omz = tpool.tile([128, NY], F32, tag="omz")
                    nc.vector.tensor_scalar(
                        omz[:], rz[:, NY:128], -1.0, 1.0,
                        mybir.AluOpType.mult, mybir.AluOpType.add,
                    )
                    # --- tails per n-half; half 0 overlaps half 1's matmuls,
                    # half 1 overlaps the next step's first k-half ---
                    NH = NY // 2
                    prev_y = None
                    for j, ps_n in ((0, ps_n0), (1, ps_n1)):
                        sl = slice(j * NH, (j + 1) * NH)
                        u = tpool.tile([128, NH], F32, tag=f"u{j}")
                        iu = nc.vector.tensor_mul(
                            u[:], rz[:, j * NH : (j + 1) * NH], ps_n[:]
                        )
                        if prev_y is not None:
                            # keep DVE program order: half-1 chain strictly
                            # after half-0's, so half-0 finishes early
                            add_dep_helper(
                                iu.ins, prev_y.ins, sync=False,
                                reason="pipeline n-halves",
                            )
                        v = tpool.tile([128, NH], F32, tag=f"v{j}")
                        nc.vector.tensor_add(
                            v[:], u[:], gis[:, 128 + j * NH : 128 + (j + 1) * NH]
                        )
                        nst = tpool.tile([128, NH], F32, tag=f"nst{j}")
                        itn = nc.scalar.activation(nst[:], v[:], TANH)
                        h1 = tpool.tile([128, NH], F32, tag=f"h1{j}")
                        nc.vector.tensor_mul(h1[:], nst[:], omz[:, sl])
                        # bf16 h half for next step's matmuls (output-cast add)
                        ib = nc.vector.tensor_add(hTb[:, sl], h1[:], zh[:, sl])
                        # fp32 h (= y_t) off the critical path
                        iy = nc.vector.tensor_add(
                            y_stage[:, t * NY + j * NH : t * NY + (j + 1) * NH],
                            h1[:],
                            zh[:, sl],
                        )
                        add_dep_helper(
                            iy.ins, ib.ins, sync=False, reason="y after hTb"
                        )
                        prev_y = iy
                    prev_step_y = prev_y
                    prev_step_tanh = itn

                nc.vector.tensor_copy(hTf[:], y_stage[:, (TCR - 1) * NY :])
                nc.sync.dma_start(
                    y_raw[:, bass.ds(ci * (TCR * NY), TCR * NY)], y_stage[:]
                )
            nc.sync.dma_start(hx_raw[:], hTf[:])
    nc.compile()
    return nc


_CACHE = {}


def _prep_inputs(x, h0, W_ih, W_hh, b_ih):
    bf = ml_dtypes.bfloat16
    x = np.ascontiguousarray(np.asarray(x, dtype=np.float32))
    h0 = np.asarray(h0, dtype=np.float32)
    W_ih = np.asarray(W_ih, dtype=np.float32)
    W_hh = np.asarray(W_hh, dtype=np.float32)
    b_ih = np.asarray(b_ih, dtype=np.float32)

    # shared weights
    wihT = np.ascontiguousarray(W_ih.T).reshape(ICT, 128, G3).astype(bf)
    whhT = np.ascontiguousarray(W_hh.T).reshape(KT, 128, G3).astype(bf)
    b_arr = np.ascontiguousarray(b_ih.reshape(MT, 128).T)

    in_maps = []
    for c in range(NC):
        x_c = x[c * BL : (c + 1) * BL]                      # [BL, S, I]
        xT_c = np.ascontiguousarray(x_c.transpose(2, 1, 0)) # [I, S, BL]
        xT_c = xT_c.reshape(ICT, 128, S, BL).astype(bf)
        h0_c = h0[c * BL : (c + 1) * BL]                    # [BL, H]
        h0T_c = np.ascontiguousarray(
            h0_c.reshape(BL, KT, 128).transpose(2, 1, 0).reshape(128, KT * BL)
        )
        in_maps.append(
            {"xT": xT_c, "h0T": h0T_c, "WihT": wihT, "WhhT": whhT, "bih": b_arr}
        )
    return in_maps


def _postprocess(results):
    y_full = np.empty((B, S, H), dtype=np.float32)
    hx_full = np.empty((B, H), dtype=np.float32)
    for c in range(NC):
        y_raw = results[c]["y_raw"]        # [128, S*KT*BL]
        hx_raw = results[c]["hx_raw"]      # [128, KT*BL]
        y = y_raw.reshape(128, S, KT, BL).transpose(3, 1, 2, 0).reshape(BL, S, H)
        y_full[c * BL : (c + 1) * BL] = y
        hx = hx_raw.reshape(128, KT, BL).transpose(2, 1, 0).reshape(BL, H)
        hx_full[c * BL : (c + 1) * BL] = hx
    return y_full, hx_full


def kernel(x, h0, W_ih, W_hh, b_ih):
    if "nc" not in _CACHE:
        _CACHE["nc"] = build()
    nc = _CACHE["nc"]
    in_maps = _prep_inputs(x, h0, W_ih, W_hh, b_ih)
    trace = bool(int(os.environ.get("GRU_TRACE", "0")))
    res = run_bass_kernel_spmd(nc, in_maps, list(range(NC)), trace=trace)
    _CACHE["last_result"] = res
    return _postprocess(res.results)


if __name__ == "__main__":
    rng = np.random.default_rng(0)
    sc = 1.0 / np.sqrt(H)
    inputs = {
        "x": rng.standard_normal((B, S, I), dtype=np.float32),
        "h0": np.zeros((B, H), dtype=np.float32),
        "W_ih": (rng.standard_normal((G3, I), dtype=np.float32) * sc),
        "W_hh": (rng.standard_normal((G3, H), dtype=np.float32) * sc),
        "b_ih": (rng.standard_normal(G3, dtype=np.float32) * sc),
    }
    y, hx = kernel(**inputs)
    print("ok", y.shape, hx.shape, float(np.abs(y).max()))


# revision 18
# speedup vs baseline: 1.5620x; 1.2183x over previous
"""GRU kernel for Trainium2, 8 NeuronCores, data-parallel over batch.

Problem: B=64, S=1024, I=H=1024 GRU (bias_ih only).
  gi = x @ W_ih.T + b_ih            (big parallel GEMM)
  per step: gh = h @ W_hh.T; gates; h' = (1-z)*n + z*h   (serial, S steps)

Sharding: batch 64 -> 8 per core; weights replicated. All matmul operands
bf16 (fp32 matmul is 4x slower on PE; bf16 numerics ~3e-3 rel-l2).

Layouts (per core, BL=8 local batch): hidden dim on partitions, batch in
free dim ("transposed"), so the serial recurrence needs no on-chip
transposes and gate math uses all 128 partitions.
  hT  [128, k*8+b]  = h[b, k*128+p]           (k = H-chunk 0..7)
  ghT psum [128, m*8+b] for m-tile m (0..23 over 3H)
  gi DRAM [128, t*192 + m*8 + b]  (step slices contiguous [128,192])
x is host-pre-transposed to [ic, 128, S, BL] bf16; y is emitted in the
native [128, t*64+k*8+b] layout and un-transposed on host.
"""

import contextlib
import os
import sys

import numpy as np
import ml_dtypes

for _p in ("/opt/trn_rl_repo",):
    if _p not in sys.path:
        sys.path.insert(0, _p)

import concourse.bass as bass
import concourse.bacc as bacc
import concourse.mybir as mybir
import concourse.bass_utils as _bu
from concourse import tile
from concourse.tile import add_dep_helper
from concourse.bass_utils import run_bass_kernel_spmd

B, S, I, H = 64, 1024, 1024, 1024
NC = 8
BL = B // NC            # 8
G3 = 3 * H              # 3072
MT = G3 // 128          # 24 m-tiles over 3H
KT = H // 128           # 8 k-chunks over H
ICT = I // 128          # 8 i-chunks over I
TCG = 64                # gemm chunk (steps)
TCR = 64                # recurrence chunk (steps)
F32 = mybir.dt.float32
BF16 = mybir.dt.bfloat16
SIG = mybir.ActivationFunctionType.Sigmoid
TANH = mybir.ActivationFunctionType.Tanh

# Walrus's LDWEIGHTS optimization (fast weight load) is disabled by the
# default driver flags; the recurrence is LDW-bound so it matters here.
if int(os.environ.get("GRU_LDWOPT", "0")):
    _orig_run_command = _bu.run_command

    def _run_command_ldwopt(argv, **kw):
        argv = [
            "--enable-ldw-opt=true" if a == "--enable-ldw-opt=false" else a
            for a in argv
        ]
        return _orig_run_command(argv, **kw)

    _bu.run_command = _run_command_ldwopt


def build():
    nc = bacc.Bacc("TRN2", target_bir_lowering=False, debug=False)
    xT = nc.declare_dram_parameter("xT", [ICT, 128, S, BL], BF16, isOutput=False)
    h0T = nc.declare_dram_parameter("h0T", [128, KT * BL], F32, isOutput=False)
    WihT = nc.declare_dram_parameter("WihT", [ICT, 128, G3], BF16, isOutput=False)
    WhhT = nc.declare_dram_parameter("WhhT", [KT, 128, G3], BF16, isOutput=False)
    bih = nc.declare_dram_parameter("bih", [128, MT], F32, isOutput=False)
    y_raw = nc.declare_dram_parameter("y_raw", [128, S * KT * BL], F32, isOutput=True)
    hx_raw = nc.declare_dram_parameter("hx_raw", [128, KT * BL], F32, isOutput=True)
    gi_dram = nc.dram_tensor("gi_buf", [128, S * MT * BL], F32)

    with tile.TileContext(nc) as tc:
        # ---------------- phase 1: gi = x @ W_ih.T + b_ih -> DRAM ----------
        with (
            tc.tile_pool(name="w1", bufs=1) as wpool,
            tc.tile_pool(name="p1", bufs=4, space="PSUM") as ppool,
            tc.tile_pool(name="s1", bufs=2) as spool,
        ):
            wih_sb = wpool.tile([128, ICT * G3], BF16)
            for ic in range(ICT):
                nc.sync.dma_start(wih_sb[:, ic * G3 : (ic + 1) * G3], WihT[ic])
            b_sb = wpool.tile([128, MT], F32)
            nc.sync.dma_start(b_sb[:], bih[:])

            N1 = TCG * BL  # 512 moving columns per chunk
            for ci in range(S // TCG):
                xt_sb = spool.tile([128, ICT, N1], BF16, tag="xt")
                for ic in range(ICT):
                    nc.sync.dma_start(
                        xt_sb[:, ic, :],
                        xT[ic][:, ci * TCG : (ci + 1) * TCG, :],
                    )
                stage = spool.tile([128, TCG, MT * BL], F32, tag="gistage")
                for m in range(MT):
                    ps = ppool.tile([128, TCG, BL], F32, tag="ps1")
                    for k in range(ICT):
                        nc.tensor.matmul(
                            ps[:],
                            wih_sb[:, k * G3 + m * 128 : k * G3 + (m + 1) * 128],
                            xt_sb[:, k, :],
                            start=(k == 0),
                            stop=(k == ICT - 1),
                        )
                    # scatter into per-step layout, fused + bias
                    nc.vector.tensor_scalar_add(
                        stage[:, :, m * BL : (m + 1) * BL],
                        ps[:],
                        b_sb[:, m : m + 1],
                    )
                nc.sync.dma_start(
                    gi_dram[:, ci * (TCG * MT * BL) : (ci + 1) * (TCG * MT * BL)],
                    stage[:],
                )


    # ---------------- phase 2: recurrence (raw bass, manual sems) ----------
    # Tile attaches a sem-increment to every matmul; the EVT-sem unit
    # retires increments serially (~30 ns each), which caps the 192-MM
    # recurrence step. Raw streams use 4 PE increments per step (one per
    # accumulation group; MMs complete in pc order so last-inc is sound)
    # and attach the h-dependency wait to the MATMUL so the LDWEIGHTS
    # stream prefetches weights during the gates tail.
    NG = MT * BL            # 192 gate columns per step
    NY = KT * BL            # 64
    NH = NY // 2            # 32
    NCH = S // TCR          # chunks
    CW = TCR * NY           # y columns per chunk (4096)
    GW = TCR * NG           # gi columns per chunk (12288)

    with contextlib.ExitStack() as ctx:
        def sb(name, shape, dt):
            return ctx.enter_context(nc.sbuf_tensor(name, shape, dt))

        whh = sb("whh", [128, KT * G3], BF16)
        gi_sb = [sb(f"gi{i}", [128, GW], F32) for i in range(2)]
        y_sb = [sb(f"ysb{i}", [128, CW], F32) for i in range(2)]
        srz = [sb(f"srz{i}", [128, 128], F32) for i in range(2)]
        v0t = [sb(f"v0t{i}", [128, NH], F32) for i in range(2)]
        v1t = [sb(f"v1t{i}", [128, NH], F32) for i in range(2)]
        rzt = sb("rzt", [128, 128], F32)
        zh = sb("zht", [128, NY], F32)
        omz = sb("omzt", [128, NY], F32)
        u0t = sb("u0t", [128, NH], F32)
        u1t = sb("u1t", [128, NH], F32)
        nst0 = sb("nst0", [128, NH], F32)
        nst1 = sb("nst1", [128, NH], F32)
        hTf = sb("hTft", [128, NY], F32)
        hTb = sb("hTbt", [128, NY], BF16)
        # one full PSUM bank per tile: no bank sharing between groups
        psb = [
            ctx.enter_context(nc.psum_tensor(f"psb{i}", [128, 512], F32))
            for i in range(8)
        ]
        ps_rza = [psb[0], psb[1]]
        ps_rzb = [psb[2], psb[3]]
        ps_n0 = [psb[4], psb[5]]
        ps_n1 = [psb[6], psb[7]]

        pe_s = ctx.enter_context(nc.semaphore("pe_s"))
        ve_s = ctx.enter_context(nc.semaphore("ve_s"))
        ac_s = ctx.enter_context(nc.semaphore("ac_s"))
        gi_s = ctx.enter_context(nc.semaphore("gi_s"))
        y_s = ctx.enter_context(nc.semaphore("y_s"))

        def h_prev_ap(T):
            c, s = divmod(T, TCR)
            if T == 0:
                return hTf[:]
            if s == 0:
                return y_sb[(c - 1) % 2][:, (TCR - 1) * NY : TCR * NY]
            return y_sb[c % 2][:, (s - 1) * NY : s * NY]

        with nc.Block() as block:

            @block.sync
            def _(sync):
                for k in range(KT):
                    sync.dma_start(
                        out=whh[:, k * G3 : (k + 1) * G3], in_=WhhT[k]
                    ).then_inc(gi_s, 16)
                sync.dma_start(out=hTf[:], in_=h0T[:]).then_inc(gi_s, 16)
                for cc in (0, 1):
                    for h in (0, 1):
                        sync.dma_start(
                            out=gi_sb[cc][:, h * (GW // 2) : (h + 1) * (GW // 2)],
                            in_=gi_dram[
                                :, cc * GW + h * (GW // 2) : cc * GW + (h + 1) * (GW // 2)
                            ],
                        ).then_inc(gi_s, 16)
                for c in range(NCH):
                    Tl = TCR * c + (TCR - 1)
                    sync.wait_ge(ve_s, 6 * Tl + 7)
                    sync.dma_start(
                        out=y_raw[:, c * CW : (c + 1) * CW], in_=y_sb[c % 2][:]
                    ).then_inc(y_s, 16)
                    if c + 2 < NCH:
                        for h in (0, 1):
                            sync.dma_start(
                                out=gi_sb[c % 2][
                                    :, h * (GW // 2) : (h + 1) * (GW // 2)
                                ],
                                in_=gi_dram[
                                    :,
                                    (c + 2) * GW
                                    + h * (GW // 2) : (c + 2) * GW
                                    + (h + 1) * (GW // 2),
                                ],
                            ).then_inc(gi_s, 16)
                sync.wait_ge(y_s, 16 * NCH)
                sync.dma_start(
                    out=hx_raw[:], in_=y_sb[(NCH - 1) % 2][:, (TCR - 1) * NY :]
                ).then_inc(y_s, 16)
                sync.wait_ge(y_s, 16 * (NCH + 1))

            @block.tensor
            def _(tensor):
                def group(T, m_lo, m_hi, kr, pout, poff, wait_val):
                    first = True
                    last_inst = None
                    for m in range(m_lo, m_hi):
                        for k in kr:
                            inst = tensor.matmul(
                                pout[:, (m - poff) * BL : (m - poff + 1) * BL],
                                whh[:, k * G3 + m * 128 : k * G3 + (m + 1) * 128],
                                hTb[:, k * BL : (k + 1) * BL],
                                start=(k == kr[0]),
                                stop=(k == kr[-1]),
                            )
                            if first and wait_val is not None:
                                inst._wait_ge(ve_s, wait_val)
                            first = False
                            last_inst = inst
                    last_inst.then_inc(pe_s, 1)

                for T in range(S):
                    b = T % 2
                    wa = 1 if T == 0 else 6 * (T - 1) + 5
                    wb = 1 if T == 0 else 6 * (T - 1) + 6
                    group(T, 0, 16, [0, 1, 2, 3], ps_rza[b], 0, wa)
                    group(T, 0, 16, [4, 5, 6, 7], ps_rzb[b], 0, wb)
                    group(T, 16, 20, [0, 1, 2, 3, 4, 5, 6, 7], ps_n0[b], 16, None)
                    group(T, 20, 24, [0, 1, 2, 3, 4, 5, 6, 7], ps_n1[b], 20, None)

            @block.vector
            def _(vector):
                vector.wait_ge(gi_s, 144)
                vector.tensor_copy(hTb[:], hTf[:]).then_inc(ve_s, 1)
                for T in range(S):
                    c, s = divmod(T, TCR)
                    b = T % 2
                    gis = gi_sb[c % 2][:, s * NG : (s + 1) * NG]
                    if s == 0 and c >= 2:
                        vector.wait_ge(y_s, 16 * (c - 1))
                    if s == 0:
                        vector.wait_ge(gi_s, 160 + 32 * c)
                    if s == TCR // 2:
                        vector.wait_ge(gi_s, 176 + 32 * c)
                    vector.tensor_add(
                        srz[b][:], ps_rza[b][:, 0:128], gis[:, 0:128]
                    )._wait_ge(pe_s, 4 * T + 1)
                    vector.tensor_add(srz[b][:], srz[b][:], ps_rzb[b][:, 0:128])._wait_ge(
                        pe_s, 4 * T + 2
                    ).then_inc(ve_s, 1)
                    vector.tensor_mul(zh[:], rzt[:, NY:128], h_prev_ap(T))._wait_ge(
                        ac_s, 3 * T + 1
                    )
                    vector.tensor_scalar(
                        omz[:], rzt[:, NY:128], -1.0, 1.0,
                        mybir.AluOpType.mult, mybir.AluOpType.add,
                    )
                    vector.tensor_mul(u0t[:], rzt[:, 0:NH], ps_n0[b][:, 0:NH])._wait_ge(
                        pe_s, 4 * T + 3
                    )
                    vector.tensor_add(
                        v0t[b][:], u0t[:], gis[:, 128 : 128 + NH]
                    ).then_inc(ve_s, 1)
                    vector.tensor_mul(
                        u1t[:], rzt[:, NH:NY], ps_n1[b][:, 0:NH]
                    )._wait_ge(pe_s, 4 * T + 4)
                    vector.tensor_add(
                        v1t[b][:], u1t[:], gis[:, 128 + NH : 128 + NY]
                    ).then_inc(ve_s, 1)
                    vector.tensor_mul(u0t[:], nst0[:], omz[:, 0:NH])._wait_ge(
                        ac_s, 3 * T + 2
                    )
                    vector.tensor_add(hTb[:, 0:NH], u0t[:], zh[:, 0:NH]).then_inc(
                        ve_s, 1
                    )
                    vector.tensor_add(
                        y_sb[c % 2][:, s * NY : s * NY + NH], u0t[:], zh[:, 0:NH]
                    )
                    vector.tensor_mul(u1t[:], nst1[:], omz[:, NH:NY])._wait_ge(
                        ac_s, 3 * T + 3
                    )
                    vector.tensor_add(hTb[:, NH:NY], u1t[:], zh[:, NH:NY]).then_inc(
                        ve_s, 1
                    )
                    vector.tensor_add(
                        y_sb[c % 2][:, s * NY + NH : (s + 1) * NY],
                        u1t[:],
                        zh[:, NH:NY],
                    ).then_inc(ve_s, 1)

            @block.scalar
            def _(scalar):
                for T in range(S):
                    b = T % 2
                    scalar.activation(rzt[:], srz[b][:], SIG)._wait_ge(
                        ve_s, 6 * T + 2
                    ).then_inc(ac_s, 1)
                    scalar.activation(nst0[:], v0t[b][:], TANH)._wait_ge(
                        ve_s, 6 * T + 3
                    ).then_inc(ac_s, 1)
                    scalar.activation(nst1[:], v1t[b][:], TANH)._wait_ge(
                        ve_s, 6 * T + 4
                    ).then_inc(ac_s, 1)

    nc.compile()
    return nc


_CACHE = {}


def _prep_inputs(x, h0, W_ih, W_hh, b_ih):
    bf = ml_dtypes.bfloat16
    x = np.ascontiguousarray(np.asarray(x, dtype=np.float32))
    h0 = np.asarray(h0, dtype=np.float32)
    W_ih = np.asarray(W_ih, dtype=np.float32)
    W_hh = np.asarray(W_hh, dtype=np.float32)
    b_ih = np.asarray(b_ih, dtype=np.float32)

    # shared weights
    wihT = np.ascontiguousarray(W_ih.T).reshape(ICT, 128, G3).astype(bf)
    whhT = np.ascontiguousarray(W_hh.T).reshape(KT, 128, G3).astype(bf)
    b_arr = np.ascontiguousarray(b_ih.reshape(MT, 128).T)

    in_maps = []
    for c in range(NC):
        x_c = x[c * BL : (c + 1) * BL]                      # [BL, S, I]
        xT_c = np.ascontiguousarray(x_c.transpose(2, 1, 0)) # [I, S, BL]
        xT_c = xT_c.reshape(ICT, 128, S, BL).astype(bf)
        h0_c = h0[c * BL : (c + 1) * BL]                    # [BL, H]
        h0T_c = np.ascontiguousarray(
            h0_c.reshape(BL, KT, 128).transpose(2, 1, 0).reshape(128, KT * BL)
        )
        in_maps.append(
            {"xT": xT_c, "h0T": h0T_c, "WihT": wihT, "WhhT": whhT, "bih": b_arr}
        )
    return in_maps


def _postprocess(results):
    y_full = np.empty((B, S, H), dtype=np.float32)
    hx_full = np.empty((B, H), dtype=np.float32)
    for c in range(NC):
        y_raw = results[c]["y_raw"]        # [128, S*KT*BL]
        hx_raw = results[c]["hx_raw"]      # [128, KT*BL]
        y = y_raw.reshape(128, S, KT, BL).transpose(3, 1, 2, 0).reshape(BL, S, H)
        y_full[c * BL : (c + 1) * BL] = y
        hx = hx_raw.reshape(128, KT, BL).transpose(2, 1, 0).reshape(BL, H)
        hx_full[c * BL : (c + 1) * BL] = hx
    return y_full, hx_full


def kernel(x, h0, W_ih, W_hh, b_ih):
    if "nc" not in _CACHE:
        _CACHE["nc"] = build()
    nc = _CACHE["nc"]
    in_maps = _prep_inputs(x, h0, W_ih, W_hh, b_ih)
    trace = bool(int(os.environ.get("GRU_TRACE", "0")))
    res = run_bass_kernel_spmd(nc, in_maps, list(range(NC)), trace=trace)
    _CACHE["last_result"] = res
    return _postprocess(res.results)


if __name__ == "__main__":
    rng = np.random.default_rng(0)
    sc = 1.0 / np.sqrt(H)
    inputs = {
        "x": rng.standard_normal((B, S, I), dtype=np.float32),
        "h0": np.zeros((B, H), dtype=np.float32),
        "W_ih": (rng.standard_normal((G3, I), dtype=np.float32) * sc),
        "W_hh": (rng.standard_normal((G3, H), dtype=np.float32) * sc),
        "b_ih": (rng.standard_normal(G3, dtype=np.float32) * sc),
    }
    y, hx = kernel(**inputs)
    print("ok", y.shape, hx.shape, float(np.abs(y).max()))


# revision 19
# speedup vs baseline: 1.5815x; 1.0125x over previous
"""GRU kernel for Trainium2, 8 NeuronCores, data-parallel over batch.

Problem: B=64, S=1024, I=H=1024 GRU (bias_ih only).
  gi = x @ W_ih.T + b_ih            (big parallel GEMM)
  per step: gh = h @ W_hh.T; gates; h' = (1-z)*n + z*h   (serial, S steps)

Sharding: batch 64 -> 8 per core; weights replicated. All matmul operands
bf16 (fp32 matmul is 4x slower on PE; bf16 numerics ~3e-3 rel-l2).

Layouts (per core, BL=8 local batch): hidden dim on partitions, batch in
free dim ("transposed"), so the serial recurrence needs no on-chip
transposes and gate math uses all 128 partitions.
  hT  [128, k*8+b]  = h[b, k*128+p]           (k = H-chunk 0..7)
  ghT psum [128, m*8+b] for m-tile m (0..23 over 3H)
  gi DRAM [128, t*192 + m*8 + b]  (step slices contiguous [128,192])
x is host-pre-transposed to [ic, 128, S, BL] bf16; y is emitted in the
native [128, t*64+k*8+b] layout and un-transposed on host.
"""

import contextlib
import os
import sys

import numpy as np
import ml_dtypes

for _p in ("/opt/trn_rl_repo",):
    if _p not in sys.path:
        sys.path.insert(0, _p)

import concourse.bass as bass
import concourse.bacc as bacc
import concourse.mybir as mybir
import concourse.bass_utils as _bu
from concourse import tile
from concourse.tile import add_dep_helper
from concourse.bass_utils import run_bass_kernel_spmd

B, S, I, H = 64, 1024, 1024, 1024
NC = 8
BL = B // NC            # 8
G3 = 3 * H              # 3072
MT = G3 // 128          # 24 m-tiles over 3H
KT = H // 128           # 8 k-chunks over H
ICT = I // 128          # 8 i-chunks over I
TCG = 64                # gemm chunk (steps)
TCR = 64                # recurrence chunk (steps)
F32 = mybir.dt.float32
BF16 = mybir.dt.bfloat16
SIG = mybir.ActivationFunctionType.Sigmoid
TANH = mybir.ActivationFunctionType.Tanh

# Walrus's LDWEIGHTS optimization (fast weight load) is disabled by the
# default driver flags; the recurrence is LDW-bound so it matters here.
if int(os.environ.get("GRU_LDWOPT", "0")):
    _orig_run_command = _bu.run_command

    def _run_command_ldwopt(argv, **kw):
        argv = [
            "--enable-ldw-opt=true" if a == "--enable-ldw-opt=false" else a
            for a in argv
        ]
        return _orig_run_command(argv, **kw)

    _bu.run_command = _run_command_ldwopt


def build():
    nc = bacc.Bacc("TRN2", target_bir_lowering=False, debug=False)
    xT = nc.declare_dram_parameter("xT", [ICT, 128, S, BL], BF16, isOutput=False)
    h0T = nc.declare_dram_parameter("h0T", [128, KT * BL], F32, isOutput=False)
    WihT = nc.declare_dram_parameter("WihT", [ICT, 128, G3], BF16, isOutput=False)
    WhhT = nc.declare_dram_parameter("WhhT", [KT, 128, G3], BF16, isOutput=False)
    bih = nc.declare_dram_parameter("bih", [128, MT], F32, isOutput=False)
    y_raw = nc.declare_dram_parameter("y_raw", [128, S * KT * BL], F32, isOutput=True)
    hx_raw = nc.declare_dram_parameter("hx_raw", [128, KT * BL], F32, isOutput=True)
    gi_dram = nc.dram_tensor("gi_buf", [128, S * MT * BL], F32)

    with tile.TileContext(nc) as tc:
        # ---------------- phase 1: gi = x @ W_ih.T + b_ih -> DRAM ----------
        with (
            tc.tile_pool(name="w1", bufs=1) as wpool,
            tc.tile_pool(name="p1", bufs=4, space="PSUM") as ppool,
            tc.tile_pool(name="s1", bufs=2) as spool,
        ):
            wih_sb = wpool.tile([128, ICT * G3], BF16)
            for ic in range(ICT):
                nc.sync.dma_start(wih_sb[:, ic * G3 : (ic + 1) * G3], WihT[ic])
            b_sb = wpool.tile([128, MT], F32)
            nc.sync.dma_start(b_sb[:], bih[:])

            N1 = TCG * BL  # 512 moving columns per chunk
            for ci in range(S // TCG):
                xt_sb = spool.tile([128, ICT, N1], BF16, tag="xt")
                for ic in range(ICT):
                    nc.sync.dma_start(
                        xt_sb[:, ic, :],
                        xT[ic][:, ci * TCG : (ci + 1) * TCG, :],
                    )
                stage = spool.tile([128, TCG, MT * BL], F32, tag="gistage")
                for m in range(MT):
                    ps = ppool.tile([128, TCG, BL], F32, tag="ps1")
                    for k in range(ICT):
                        nc.tensor.matmul(
                            ps[:],
                            wih_sb[:, k * G3 + m * 128 : k * G3 + (m + 1) * 128],
                            xt_sb[:, k, :],
                            start=(k == 0),
                            stop=(k == ICT - 1),
                        )
                    # scatter into per-step layout, fused + bias
                    nc.vector.tensor_scalar_add(
                        stage[:, :, m * BL : (m + 1) * BL],
                        ps[:],
                        b_sb[:, m : m + 1],
                    )
                nc.sync.dma_start(
                    gi_dram[:, ci * (TCG * MT * BL) : (ci + 1) * (TCG * MT * BL)],
                    stage[:],
                )


    # ---------------- phase 2: recurrence (raw bass, manual sems) ----------
    # Tile attaches a sem-increment to every matmul; the EVT-sem unit
    # retires increments serially (~30 ns each), which caps the 192-MM
    # recurrence step. Raw streams use 4 PE increments per step (one per
    # accumulation group; MMs complete in pc order so last-inc is sound)
    # and attach the h-dependency wait to the MATMUL so the LDWEIGHTS
    # stream prefetches weights during the gates tail.
    NG = MT * BL            # 192 gate columns per step
    NY = KT * BL            # 64
    NH = NY // 2            # 32
    NCH = S // TCR          # chunks
    CW = TCR * NY           # y columns per chunk (4096)
    GW = TCR * NG           # gi columns per chunk (12288)

    with contextlib.ExitStack() as ctx:
        def sb(name, shape, dt):
            return ctx.enter_context(nc.sbuf_tensor(name, shape, dt))

        whh = sb("whh", [128, KT * G3], BF16)
        gi_sb = [sb(f"gi{i}", [128, GW], F32) for i in range(2)]
        y_sb = [sb(f"ysb{i}", [128, CW], F32) for i in range(2)]
        srz = [sb(f"srz{i}", [128, 128], F32) for i in range(2)]
        v0t = [sb(f"v0t{i}", [128, NH], F32) for i in range(2)]
        v1t = [sb(f"v1t{i}", [128, NH], F32) for i in range(2)]
        rzt = sb("rzt", [128, 128], F32)
        zh = sb("zht", [128, NY], F32)
        omz = sb("omzt", [128, NY], F32)
        u0t = sb("u0t", [128, NH], F32)
        u1t = sb("u1t", [128, NH], F32)
        nst0 = sb("nst0", [128, NH], F32)
        nst1 = sb("nst1", [128, NH], F32)
        hTf = sb("hTft", [128, NY], F32)
        hTb = sb("hTbt", [128, NY], BF16)
        # one full PSUM bank per tile: no bank sharing between groups
        psb = [
            ctx.enter_context(nc.psum_tensor(f"psb{i}", [128, 512], F32))
            for i in range(8)
        ]
        ps_rza = [psb[0], psb[1]]
        ps_rzb = [psb[2], psb[3]]
        ps_n0 = [psb[4], psb[5]]
        ps_n1 = [psb[6], psb[7]]

        pe_s = ctx.enter_context(nc.semaphore("pe_s"))
        ve_s = ctx.enter_context(nc.semaphore("ve_s"))
        ac_s = ctx.enter_context(nc.semaphore("ac_s"))
        gi_s = ctx.enter_context(nc.semaphore("gi_s"))
        y_s = ctx.enter_context(nc.semaphore("y_s"))

        def h_prev_ap(T):
            c, s = divmod(T, TCR)
            if T == 0:
                return hTf[:]
            if s == 0:
                return y_sb[(c - 1) % 2][:, (TCR - 1) * NY : TCR * NY]
            return y_sb[c % 2][:, (s - 1) * NY : s * NY]

        with nc.Block() as block:

            @block.sync
            def _(sync):
                for k in range(KT):
                    sync.dma_start(
                        out=whh[:, k * G3 : (k + 1) * G3], in_=WhhT[k]
                    ).then_inc(gi_s, 16)
                sync.dma_start(out=hTf[:], in_=h0T[:]).then_inc(gi_s, 16)
                for cc in (0, 1):
                    for h in (0, 1):
                        sync.dma_start(
                            out=gi_sb[cc][:, h * (GW // 2) : (h + 1) * (GW // 2)],
                            in_=gi_dram[
                                :, cc * GW + h * (GW // 2) : cc * GW + (h + 1) * (GW // 2)
                            ],
                        ).then_inc(gi_s, 16)
                for c in range(NCH):
                    Tl = TCR * c + (TCR - 1)
                    sync.wait_ge(ve_s, 6 * Tl + 7)
                    sync.dma_start(
                        out=y_raw[:, c * CW : (c + 1) * CW], in_=y_sb[c % 2][:]
                    ).then_inc(y_s, 16)
                    if c + 2 < NCH:
                        for h in (0, 1):
                            sync.dma_start(
                                out=gi_sb[c % 2][
                                    :, h * (GW // 2) : (h + 1) * (GW // 2)
                                ],
                                in_=gi_dram[
                                    :,
                                    (c + 2) * GW
                                    + h * (GW // 2) : (c + 2) * GW
                                    + (h + 1) * (GW // 2),
                                ],
                            ).then_inc(gi_s, 16)
                sync.wait_ge(y_s, 16 * NCH)
                sync.dma_start(
                    out=hx_raw[:], in_=y_sb[(NCH - 1) % 2][:, (TCR - 1) * NY :]
                ).then_inc(y_s, 16)
                sync.wait_ge(y_s, 16 * (NCH + 1))

            @block.tensor
            def _(tensor):
                def group(T, m_lo, m_hi, kr, pout, poff, wait_val):
                    first = True
                    last_inst = None
                    for m in range(m_lo, m_hi):
                        for k in kr:
                            inst = tensor.matmul(
                                pout[:, (m - poff) * BL : (m - poff + 1) * BL],
                                whh[:, k * G3 + m * 128 : k * G3 + (m + 1) * 128],
                                hTb[:, k * BL : (k + 1) * BL],
                                start=(k == kr[0]),
                                stop=(k == kr[-1]),
                            )
                            if first and wait_val is not None:
                                inst._wait_ge(ve_s, wait_val)
                            first = False
                            last_inst = inst
                    last_inst.then_inc(pe_s, 1)

                for T in range(S):
                    b = T % 2
                    wa = 1 if T == 0 else 6 * (T - 1) + 4
                    wb = 1 if T == 0 else 6 * (T - 1) + 6
                    group(T, 0, 16, [0, 1, 2, 3], ps_rza[b], 0, wa)
                    group(T, 0, 16, [4, 5, 6, 7], ps_rzb[b], 0, wb)
                    group(T, 16, 20, [0, 1, 2, 3, 4, 5, 6, 7], ps_n0[b], 16, None)
                    group(T, 20, 24, [0, 1, 2, 3, 4, 5, 6, 7], ps_n1[b], 20, None)

            @block.vector
            def _(vector):
                vector.wait_ge(gi_s, 144)
                vector.tensor_copy(hTb[:], hTf[:]).then_inc(ve_s, 1)
                for T in range(S):
                    c, s = divmod(T, TCR)
                    b = T % 2
                    gis = gi_sb[c % 2][:, s * NG : (s + 1) * NG]
                    if s == 0 and c >= 2:
                        vector.wait_ge(y_s, 16 * (c - 1))
                    if s == 0:
                        vector.wait_ge(gi_s, 160 + 32 * c)
                    if s == TCR // 2:
                        vector.wait_ge(gi_s, 176 + 32 * c)
                    vector.tensor_add(
                        srz[b][:], ps_rza[b][:, 0:128], gis[:, 0:128]
                    )._wait_ge(pe_s, 4 * T + 1)
                    vector.tensor_add(srz[b][:], srz[b][:], ps_rzb[b][:, 0:128])._wait_ge(
                        pe_s, 4 * T + 2
                    ).then_inc(ve_s, 1)
                    vector.tensor_mul(zh[:], rzt[:, NY:128], h_prev_ap(T))._wait_ge(
                        ac_s, 3 * T + 1
                    )
                    vector.tensor_scalar(
                        omz[:], rzt[:, NY:128], -1.0, 1.0,
                        mybir.AluOpType.mult, mybir.AluOpType.add,
                    )
                    vector.tensor_mul(u0t[:], rzt[:, 0:NH], ps_n0[b][:, 0:NH])._wait_ge(
                        pe_s, 4 * T + 3
                    )
                    vector.tensor_add(
                        v0t[b][:], u0t[:], gis[:, 128 : 128 + NH]
                    ).then_inc(ve_s, 1)
                    # h10 precomputed so hTb0 (whose real dependency is the
                    # all-matmuls-complete WAR on hTb) fires immediately
                    h10 = tpl0 = u0t  # reuse
                    vector.tensor_mul(h10[:], nst0[:], omz[:, 0:NH])._wait_ge(
                        ac_s, 3 * T + 2
                    )
                    vector.tensor_add(hTb[:, 0:NH], h10[:], zh[:, 0:NH])._wait_ge(
                        pe_s, 4 * T + 4
                    ).then_inc(ve_s, 1)
                    vector.tensor_mul(u1t[:], rzt[:, NH:NY], ps_n1[b][:, 0:NH])
                    vector.tensor_add(
                        v1t[b][:], u1t[:], gis[:, 128 + NH : 128 + NY]
                    ).then_inc(ve_s, 1)
                    vector.tensor_mul(u1t[:], nst1[:], omz[:, NH:NY])._wait_ge(
                        ac_s, 3 * T + 3
                    )
                    vector.tensor_add(hTb[:, NH:NY], u1t[:], zh[:, NH:NY]).then_inc(
                        ve_s, 1
                    )
                    vector.tensor_add(
                        y_sb[c % 2][:, s * NY : s * NY + NH], h10[:], zh[:, 0:NH]
                    )
                    vector.tensor_add(
                        y_sb[c % 2][:, s * NY + NH : (s + 1) * NY],
                        u1t[:],
                        zh[:, NH:NY],
                    ).then_inc(ve_s, 1)

            @block.scalar
            def _(scalar):
                for T in range(S):
                    b = T % 2
                    scalar.activation(rzt[:], srz[b][:], SIG)._wait_ge(
                        ve_s, 6 * T + 2
                    ).then_inc(ac_s, 1)
                    scalar.activation(nst0[:], v0t[b][:], TANH)._wait_ge(
                        ve_s, 6 * T + 3
                    ).then_inc(ac_s, 1)
                    scalar.activation(nst1[:], v1t[b][:], TANH)._wait_ge(
                        ve_s, 6 * T + 5
                    ).then_inc(ac_s, 1)

    nc.compile()
    return nc


_CACHE = {}


def _prep_inputs(x, h0, W_ih, W_hh, b_ih):
    bf = ml_dtypes.bfloat16
    x = np.ascontiguousarray(np.asarray(x, dtype=np.float32))
    h0 = np.asarray(h0, dtype=np.float32)
    W_ih = np.asarray(W_ih, dtype=np.float32)
    W_hh = np.asarray(W_hh, dtype=np.float32)
    b_ih = np.asarray(b_ih, dtype=np.float32)

    # shared weights
    wihT = np.ascontiguousarray(W_ih.T).reshape(ICT, 128, G3).astype(bf)
    whhT = np.ascontiguousarray(W_hh.T).reshape(KT, 128, G3).astype(bf)
    b_arr = np.ascontiguousarray(b_ih.reshape(MT, 128).T)

    in_maps = []
    for c in range(NC):
        x_c = x[c * BL : (c + 1) * BL]                      # [BL, S, I]
        xT_c = np.ascontiguousarray(x_c.transpose(2, 1, 0)) # [I, S, BL]
        xT_c = xT_c.reshape(ICT, 128, S, BL).astype(bf)
        h0_c = h0[c * BL : (c + 1) * BL]                    # [BL, H]
        h0T_c = np.ascontiguousarray(
            h0_c.reshape(BL, KT, 128).transpose(2, 1, 0).reshape(128, KT * BL)
        )
        in_maps.append(
            {"xT": xT_c, "h0T": h0T_c, "WihT": wihT, "WhhT": whhT, "bih": b_arr}
        )
    return in_maps


def _postprocess(results):
    y_full = np.empty((B, S, H), dtype=np.float32)
    hx_full = np.empty((B, H), dtype=np.float32)
    for c in range(NC):
        y_raw = results[c]["y_raw"]        # [128, S*KT*BL]
        hx_raw = results[c]["hx_raw"]      # [128, KT*BL]
        y = y_raw.reshape(128, S, KT, BL).transpose(3, 1, 2, 0).reshape(BL, S, H)
        y_full[c * BL : (c + 1) * BL] = y
        hx = hx_raw.reshape(128, KT, BL).transpose(2, 1, 0).reshape(BL, H)
        hx_full[c * BL : (c + 1) * BL] = hx
    return y_full, hx_full


def kernel(x, h0, W_ih, W_hh, b_ih):
    if "nc" not in _CACHE:
        _CACHE["nc"] = build()
    nc = _CACHE["nc"]
    in_maps = _prep_inputs(x, h0, W_ih, W_hh, b_ih)
    trace = bool(int(os.environ.get("GRU_TRACE", "0")))
    res = run_bass_kernel_spmd(nc, in_maps, list(range(NC)), trace=trace)
    _CACHE["last_result"] = res
    return _postprocess(res.results)


if __name__ == "__main__":
    rng = np.random.default_rng(0)
    sc = 1.0 / np.sqrt(H)
    inputs = {
        "x": rng.standard_normal((B, S, I), dtype=np.float32),
        "h0": np.zeros((B, H), dtype=np.float32),
        "W_ih": (rng.standard_normal((G3, I), dtype=np.float32) * sc),
        "W_hh": (rng.standard_normal((G3, H), dtype=np.float32) * sc),
        "b_ih": (rng.standard_normal(G3, dtype=np.float32) * sc),
    }
    y, hx = kernel(**inputs)
    print("ok", y.shape, hx.shape, float(np.abs(y).max()))


# revision 21
# speedup vs baseline: 1.6003x; 1.0119x over previous
"""GRU kernel for Trainium2, 8 NeuronCores, data-parallel over batch.

Problem: B=64, S=1024, I=H=1024 GRU (bias_ih only).
  gi = x @ W_ih.T + b_ih            (big parallel GEMM)
  per step: gh = h @ W_hh.T; gates; h' = (1-z)*n + z*h   (serial, S steps)

Sharding: batch 64 -> 8 per core; weights replicated. All matmul operands
bf16 (fp32 matmul is 4x slower on PE; bf16 numerics ~3e-3 rel-l2).

Layouts (per core, BL=8 local batch): hidden dim on partitions, batch in
free dim ("transposed"), so the serial recurrence needs no on-chip
transposes and gate math uses all 128 partitions.
  hT  [128, k*8+b]  = h[b, k*128+p]           (k = H-chunk 0..7)
  ghT psum [128, m*8+b] for m-tile m (0..23 over 3H)
  gi DRAM [128, t*192 + m*8 + b]  (step slices contiguous [128,192])
x is host-pre-transposed to [ic, 128, S, BL] bf16; y is emitted in the
native [128, t*64+k*8+b] layout and un-transposed on host.
"""

import contextlib
import os
import sys

import numpy as np
import ml_dtypes

for _p in ("/opt/trn_rl_repo",):
    if _p not in sys.path:
        sys.path.insert(0, _p)

import concourse.bass as bass
import concourse.bacc as bacc
import concourse.mybir as mybir
import concourse.bass_utils as _bu
from concourse import tile
from concourse.tile import add_dep_helper
from concourse.bass_utils import run_bass_kernel_spmd

B, S, I, H = 64, 1024, 1024, 1024
NC = 8
BL = B // NC            # 8
G3 = 3 * H              # 3072
MT = G3 // 128          # 24 m-tiles over 3H
KT = H // 128           # 8 k-chunks over H
ICT = I // 128          # 8 i-chunks over I
TCG = 64                # gemm chunk (steps)
TCR = 64                # recurrence chunk (steps)
F32 = mybir.dt.float32
BF16 = mybir.dt.bfloat16
SIG = mybir.ActivationFunctionType.Sigmoid
TANH = mybir.ActivationFunctionType.Tanh

# Walrus's LDWEIGHTS optimization (fast weight load) is disabled by the
# default driver flags; the recurrence is LDW-bound so it matters here.
if int(os.environ.get("GRU_LDWOPT", "0")):
    _orig_run_command = _bu.run_command

    def _run_command_ldwopt(argv, **kw):
        argv = [
            "--enable-ldw-opt=true" if a == "--enable-ldw-opt=false" else a
            for a in argv
        ]
        return _orig_run_command(argv, **kw)

    _bu.run_command = _run_command_ldwopt


def build():
    nc = bacc.Bacc("TRN2", target_bir_lowering=False, debug=False)
    xT = nc.declare_dram_parameter("xT", [ICT, 128, S, BL], BF16, isOutput=False)
    h0T = nc.declare_dram_parameter("h0T", [128, KT * BL], F32, isOutput=False)
    WihT = nc.declare_dram_parameter("WihT", [ICT, 128, G3], BF16, isOutput=False)
    WhhT = nc.declare_dram_parameter("WhhT", [KT, 128, G3], BF16, isOutput=False)
    bih = nc.declare_dram_parameter("bih", [128, MT], F32, isOutput=False)
    y_raw = nc.declare_dram_parameter("y_raw", [128, S * KT * BL], F32, isOutput=True)
    hx_raw = nc.declare_dram_parameter("hx_raw", [128, KT * BL], F32, isOutput=True)
    gi_dram = nc.dram_tensor("gi_buf", [128, S * MT * BL], F32)

    with tile.TileContext(nc) as tc:
        # ---------------- phase 1: gi = x @ W_ih.T + b_ih -> DRAM ----------
        with (
            tc.tile_pool(name="w1", bufs=1) as wpool,
            tc.tile_pool(name="p1", bufs=4, space="PSUM") as ppool,
            tc.tile_pool(name="s1", bufs=2) as spool,
            tc.tile_pool(name="sx", bufs=3) as xpool,
        ):
            wih_sb = wpool.tile([128, ICT * G3], BF16)
            for ic in range(ICT):
                nc.sync.dma_start(wih_sb[:, ic * G3 : (ic + 1) * G3], WihT[ic])
            b_sb = wpool.tile([128, MT], F32)
            nc.sync.dma_start(b_sb[:], bih[:])

            N1 = TCG * BL  # 512 moving columns per chunk
            for ci in range(S // TCG):
                xt_sb = xpool.tile([128, ICT, N1], BF16, tag="xt")
                for ic in range(ICT):
                    nc.sync.dma_start(
                        xt_sb[:, ic, :],
                        xT[ic][:, ci * TCG : (ci + 1) * TCG, :],
                    )
                stage = spool.tile([128, TCG, MT * BL], F32, tag="gistage")
                for m in range(MT):
                    ps = ppool.tile([128, TCG, BL], F32, tag="ps1")
                    for k in range(ICT):
                        nc.tensor.matmul(
                            ps[:],
                            wih_sb[:, k * G3 + m * 128 : k * G3 + (m + 1) * 128],
                            xt_sb[:, k, :],
                            start=(k == 0),
                            stop=(k == ICT - 1),
                        )
                    # scatter into per-step layout, fused + bias
                    nc.vector.tensor_scalar_add(
                        stage[:, :, m * BL : (m + 1) * BL],
                        ps[:],
                        b_sb[:, m : m + 1],
                    )
                nc.sync.dma_start(
                    gi_dram[:, ci * (TCG * MT * BL) : (ci + 1) * (TCG * MT * BL)],
                    stage[:],
                )


    # ---------------- phase 2: recurrence (raw bass, manual sems) ----------
    # Tile attaches a sem-increment to every matmul; the EVT-sem unit
    # retires increments serially (~30 ns each), which caps the 192-MM
    # recurrence step. Raw streams use 4 PE increments per step (one per
    # accumulation group; MMs complete in pc order so last-inc is sound)
    # and attach the h-dependency wait to the MATMUL so the LDWEIGHTS
    # stream prefetches weights during the gates tail.
    NG = MT * BL            # 192 gate columns per step
    NY = KT * BL            # 64
    NH = NY // 2            # 32
    NCH = S // TCR          # chunks
    CW = TCR * NY           # y columns per chunk (4096)
    GW = TCR * NG           # gi columns per chunk (12288)

    with contextlib.ExitStack() as ctx:
        def sb(name, shape, dt):
            return ctx.enter_context(nc.sbuf_tensor(name, shape, dt))

        whh = sb("whh", [128, KT * G3], BF16)
        gi_sb = [sb(f"gi{i}", [128, GW], F32) for i in range(2)]
        y_sb = [sb(f"ysb{i}", [128, CW], F32) for i in range(2)]
        srz = [sb(f"srz{i}", [128, 128], F32) for i in range(2)]
        v0t = [sb(f"v0t{i}", [128, NH], F32) for i in range(2)]
        v1t = [sb(f"v1t{i}", [128, NH], F32) for i in range(2)]
        rzt = sb("rzt", [128, 128], F32)
        zh = sb("zht", [128, NY], F32)
        omz = sb("omzt", [128, NY], F32)
        u0t = sb("u0t", [128, NH], F32)
        u1t = sb("u1t", [128, NH], F32)
        nst0 = sb("nst0", [128, NH], F32)
        nst1 = sb("nst1", [128, NH], F32)
        hTf = sb("hTft", [128, NY], F32)
        hTb = sb("hTbt", [128, NY], BF16)
        # one full PSUM bank per tile: no bank sharing between groups
        psb = [
            ctx.enter_context(nc.psum_tensor(f"psb{i}", [128, 512], F32))
            for i in range(8)
        ]
        ps_rza = [psb[0], psb[1]]
        ps_rzb = [psb[2], psb[3]]
        ps_n0 = [psb[4], psb[5]]
        ps_n1 = [psb[6], psb[7]]

        pe_s = ctx.enter_context(nc.semaphore("pe_s"))
        ve_s = ctx.enter_context(nc.semaphore("ve_s"))
        ac_s = ctx.enter_context(nc.semaphore("ac_s"))
        gi_s = ctx.enter_context(nc.semaphore("gi_s"))
        y_s = ctx.enter_context(nc.semaphore("y_s"))

        def h_prev_ap(T):
            c, s = divmod(T, TCR)
            if T == 0:
                return hTf[:]
            if s == 0:
                return y_sb[(c - 1) % 2][:, (TCR - 1) * NY : TCR * NY]
            return y_sb[c % 2][:, (s - 1) * NY : s * NY]

        with nc.Block() as block:

            @block.sync
            def _(sync):
                for k in range(KT):
                    sync.dma_start(
                        out=whh[:, k * G3 : (k + 1) * G3], in_=WhhT[k]
                    ).then_inc(gi_s, 16)
                sync.dma_start(out=hTf[:], in_=h0T[:]).then_inc(gi_s, 16)
                for cc in (0, 1):
                    for h in (0, 1):
                        sync.dma_start(
                            out=gi_sb[cc][:, h * (GW // 2) : (h + 1) * (GW // 2)],
                            in_=gi_dram[
                                :, cc * GW + h * (GW // 2) : cc * GW + (h + 1) * (GW // 2)
                            ],
                        ).then_inc(gi_s, 16)
                for c in range(NCH):
                    Tm = TCR * c + (TCR // 2 - 1)
                    Tl = TCR * c + (TCR - 1)
                    sync.wait_ge(ve_s, 6 * Tm + 7)
                    sync.dma_start(
                        out=y_raw[:, c * CW : c * CW + CW // 2],
                        in_=y_sb[c % 2][:, : CW // 2],
                    ).then_inc(y_s, 16)
                    sync.wait_ge(ve_s, 6 * Tl + 7)
                    sync.dma_start(
                        out=y_raw[:, c * CW + CW // 2 : (c + 1) * CW],
                        in_=y_sb[c % 2][:, CW // 2 :],
                    ).then_inc(y_s, 16)
                    if c + 2 < NCH:
                        for h in (0, 1):
                            sync.dma_start(
                                out=gi_sb[c % 2][
                                    :, h * (GW // 2) : (h + 1) * (GW // 2)
                                ],
                                in_=gi_dram[
                                    :,
                                    (c + 2) * GW
                                    + h * (GW // 2) : (c + 2) * GW
                                    + (h + 1) * (GW // 2),
                                ],
                            ).then_inc(gi_s, 16)
                sync.wait_ge(y_s, 32 * NCH)
                sync.dma_start(
                    out=hx_raw[:], in_=y_sb[(NCH - 1) % 2][:, (TCR - 1) * NY :]
                ).then_inc(y_s, 16)
                sync.wait_ge(y_s, 32 * NCH + 16)

            @block.tensor
            def _(tensor):
                def group(T, m_lo, m_hi, kr, pout, poff, wait_val):
                    first = True
                    last_inst = None
                    for m in range(m_lo, m_hi):
                        for k in kr:
                            inst = tensor.matmul(
                                pout[:, (m - poff) * BL : (m - poff + 1) * BL],
                                whh[:, k * G3 + m * 128 : k * G3 + (m + 1) * 128],
                                hTb[:, k * BL : (k + 1) * BL],
                                start=(k == kr[0]),
                                stop=(k == kr[-1]),
                            )
                            if first and wait_val is not None:
                                inst._wait_ge(ve_s, wait_val)
                            first = False
                            last_inst = inst
                    last_inst.then_inc(pe_s, 1)

                for T in range(S):
                    b = T % 2
                    wa = 1 if T == 0 else 6 * (T - 1) + 4
                    wb = 1 if T == 0 else 6 * (T - 1) + 6
                    group(T, 0, 16, [0, 1, 2, 3], ps_rza[b], 0, wa)
                    group(T, 0, 16, [4, 5, 6, 7], ps_rzb[b], 0, wb)
                    group(T, 16, 20, [0, 1, 2, 3, 4, 5, 6, 7], ps_n0[b], 16, None)
                    group(T, 20, 24, [0, 1, 2, 3, 4, 5, 6, 7], ps_n1[b], 20, None)

            @block.vector
            def _(vector):
                vector.wait_ge(gi_s, 144)
                vector.tensor_copy(hTb[:], hTf[:]).then_inc(ve_s, 1)
                for T in range(S):
                    c, s = divmod(T, TCR)
                    b = T % 2
                    gis = gi_sb[c % 2][:, s * NG : (s + 1) * NG]
                    if s == 0 and c >= 2:
                        vector.wait_ge(y_s, 32 * (c - 1))
                    if s == 0:
                        vector.wait_ge(gi_s, 160 + 32 * c)
                    if s == TCR // 2:
                        vector.wait_ge(gi_s, 176 + 32 * c)
                    vector.tensor_add(
                        srz[b][:], ps_rza[b][:, 0:128], gis[:, 0:128]
                    )._wait_ge(pe_s, 4 * T + 1)
                    vector.tensor_add(srz[b][:], srz[b][:], ps_rzb[b][:, 0:128])._wait_ge(
                        pe_s, 4 * T + 2
                    ).then_inc(ve_s, 1)
                    vector.tensor_mul(zh[:], rzt[:, NY:128], h_prev_ap(T))._wait_ge(
                        ac_s, 3 * T + 1
                    )
                    vector.tensor_scalar(
                        omz[:], rzt[:, NY:128], -1.0, 1.0,
                        mybir.AluOpType.mult, mybir.AluOpType.add,
                    )
                    vector.tensor_mul(u0t[:], rzt[:, 0:NH], ps_n0[b][:, 0:NH])._wait_ge(
                        pe_s, 4 * T + 3
                    )
                    vector.tensor_add(
                        v0t[b][:], u0t[:], gis[:, 128 : 128 + NH]
                    ).then_inc(ve_s, 1)
                    # h10 precomputed so hTb0 (whose real dependency is the
                    # all-matmuls-complete WAR on hTb) fires immediately
                    h10 = tpl0 = u0t  # reuse
                    vector.tensor_mul(h10[:], nst0[:], omz[:, 0:NH])._wait_ge(
                        ac_s, 3 * T + 2
                    )
                    vector.tensor_add(hTb[:, 0:NH], h10[:], zh[:, 0:NH])._wait_ge(
                        pe_s, 4 * T + 4
                    ).then_inc(ve_s, 1)
                    vector.tensor_mul(u1t[:], rzt[:, NH:NY], ps_n1[b][:, 0:NH])
                    vector.tensor_add(
                        v1t[b][:], u1t[:], gis[:, 128 + NH : 128 + NY]
                    ).then_inc(ve_s, 1)
                    vector.tensor_mul(u1t[:], nst1[:], omz[:, NH:NY])._wait_ge(
                        ac_s, 3 * T + 3
                    )
                    vector.tensor_add(hTb[:, NH:NY], u1t[:], zh[:, NH:NY]).then_inc(
                        ve_s, 1
                    )
                    vector.tensor_add(
                        y_sb[c % 2][:, s * NY : s * NY + NH], h10[:], zh[:, 0:NH]
                    )
                    vector.tensor_add(
                        y_sb[c % 2][:, s * NY + NH : (s + 1) * NY],
                        u1t[:],
                        zh[:, NH:NY],
                    ).then_inc(ve_s, 1)

            @block.scalar
            def _(scalar):
                for T in range(S):
                    b = T % 2
                    scalar.activation(rzt[:], srz[b][:], SIG)._wait_ge(
                        ve_s, 6 * T + 2
                    ).then_inc(ac_s, 1)
                    scalar.activation(nst0[:], v0t[b][:], TANH)._wait_ge(
                        ve_s, 6 * T + 3
                    ).then_inc(ac_s, 1)
                    scalar.activation(nst1[:], v1t[b][:], TANH)._wait_ge(
                        ve_s, 6 * T + 5
                    ).then_inc(ac_s, 1)

    nc.compile()
    return nc


_CACHE = {}


def _prep_inputs(x, h0, W_ih, W_hh, b_ih):
    bf = ml_dtypes.bfloat16
    x = np.ascontiguousarray(np.asarray(x, dtype=np.float32))
    h0 = np.asarray(h0, dtype=np.float32)
    W_ih = np.asarray(W_ih, dtype=np.float32)
    W_hh = np.asarray(W_hh, dtype=np.float32)
    b_ih = np.asarray(b_ih, dtype=np.float32)

    # shared weights
    wihT = np.ascontiguousarray(W_ih.T).reshape(ICT, 128, G3).astype(bf)
    whhT = np.ascontiguousarray(W_hh.T).reshape(KT, 128, G3).astype(bf)
    b_arr = np.ascontiguousarray(b_ih.reshape(MT, 128).T)

    in_maps = []
    for c in range(NC):
        x_c = x[c * BL : (c + 1) * BL]                      # [BL, S, I]
        xT_c = np.ascontiguousarray(x_c.transpose(2, 1, 0)) # [I, S, BL]
        xT_c = xT_c.reshape(ICT, 128, S, BL).astype(bf)
        h0_c = h0[c * BL : (c + 1) * BL]                    # [BL, H]
        h0T_c = np.ascontiguousarray(
            h0_c.reshape(BL, KT, 128).transpose(2, 1, 0).reshape(128, KT * BL)
        )
        in_maps.append(
            {"xT": xT_c, "h0T": h0T_c, "WihT": wihT, "WhhT": whhT, "bih": b_arr}
        )
    return in_maps


def _postprocess(results):
    y_full = np.empty((B, S, H), dtype=np.float32)
    hx_full = np.empty((B, H), dtype=np.float32)
    for c in range(NC):
        y_raw = results[c]["y_raw"]        # [128, S*KT*BL]
        hx_raw = results[c]["hx_raw"]      # [128, KT*BL]
        y = y_raw.reshape(128, S, KT, BL).transpose(3, 1, 2, 0).reshape(BL, S, H)
        y_full[c * BL : (c + 1) * BL] = y
        hx = hx_raw.reshape(128, KT, BL).transpose(2, 1, 0).reshape(BL, H)
        hx_full[c * BL : (c + 1) * BL] = hx
    return y_full, hx_full


def kernel(x, h0, W_ih, W_hh, b_ih):
    if "nc" not in _CACHE:
        _CACHE["nc"] = build()
    nc = _CACHE["nc"]
    in_maps = _prep_inputs(x, h0, W_ih, W_hh, b_ih)
    trace = bool(int(os.environ.get("GRU_TRACE", "0")))
    res = run_bass_kernel_spmd(nc, in_maps, list(range(NC)), trace=trace)
    _CACHE["last_result"] = res
    return _postprocess(res.results)


if __name__ == "__main__":
    rng = np.random.default_rng(0)
    sc = 1.0 / np.sqrt(H)
    inputs = {
        "x": rng.standard_normal((B, S, I), dtype=np.float32),
        "h0": np.zeros((B, H), dtype=np.float32),
        "W_ih": (rng.standard_normal((G3, I), dtype=np.float32) * sc),
        "W_hh": (rng.standard_normal((G3, H), dtype=np.float32) * sc),
        "b_ih": (rng.standard_normal(G3, dtype=np.float32) * sc),
    }
    y, hx = kernel(**inputs)
    print("ok", y.shape, hx.shape, float(np.abs(y).max()))


# revision 24
# speedup vs baseline: 1.6450x; 1.0279x over previous
"""GRU kernel for Trainium2, 8 NeuronCores, data-parallel over batch.

Problem: B=64, S=1024, I=H=1024 GRU (bias_ih only).
  gi = x @ W_ih.T + b_ih            (big parallel GEMM)
  per step: gh = h @ W_hh.T; gates; h' = (1-z)*n + z*h   (serial, S steps)

Sharding: batch 64 -> 8 per core; weights replicated. All matmul operands
bf16 (fp32 matmul is 4x slower on PE; bf16 numerics ~3e-3 rel-l2).

Layouts (per core, BL=8 local batch): hidden dim on partitions, batch in
free dim ("transposed"), so the serial recurrence needs no on-chip
transposes and gate math uses all 128 partitions.
  hT  [128, k*8+b]  = h[b, k*128+p]           (k = H-chunk 0..7)
  ghT psum [128, m*8+b] for m-tile m (0..23 over 3H)
  gi DRAM [128, t*192 + m*8 + b]  (step slices contiguous [128,192])
x is host-pre-transposed to [ic, 128, S, BL] bf16; y is emitted in the
native [128, t*64+k*8+b] layout and un-transposed on host.
"""

import contextlib
import os
import sys

import numpy as np
import ml_dtypes

for _p in ("/opt/trn_rl_repo",):
    if _p not in sys.path:
        sys.path.insert(0, _p)

import concourse.bass as bass
import concourse.bacc as bacc
import concourse.mybir as mybir
import concourse.bass_utils as _bu
from concourse import tile
from concourse.tile import add_dep_helper
from concourse.bass_utils import run_bass_kernel_spmd

B, S, I, H = 64, 1024, 1024, 1024
NC = 8
BL = B // NC            # 8
G3 = 3 * H              # 3072
MT = G3 // 128          # 24 m-tiles over 3H
KT = H // 128           # 8 k-chunks over H
ICT = I // 128          # 8 i-chunks over I
TCG = 64                # gemm chunk (steps)
TCR = 64                # recurrence chunk (steps)
F32 = mybir.dt.float32
BF16 = mybir.dt.bfloat16
SIG = mybir.ActivationFunctionType.Sigmoid
TANH = mybir.ActivationFunctionType.Tanh

# Walrus's LDWEIGHTS optimization (fast weight load) is disabled by the
# default driver flags; the recurrence is LDW-bound so it matters here.
if int(os.environ.get("GRU_LDWOPT", "0")):
    _orig_run_command = _bu.run_command

    def _run_command_ldwopt(argv, **kw):
        argv = [
            "--enable-ldw-opt=true" if a == "--enable-ldw-opt=false" else a
            for a in argv
        ]
        return _orig_run_command(argv, **kw)

    _bu.run_command = _run_command_ldwopt


def build():
    nc = bacc.Bacc("TRN2", target_bir_lowering=False, debug=False)
    xT = nc.declare_dram_parameter("xT", [ICT, 128, S, BL], BF16, isOutput=False)
    h0T = nc.declare_dram_parameter("h0T", [128, KT * BL], F32, isOutput=False)
    WihT = nc.declare_dram_parameter("WihT", [ICT, 128, G3], BF16, isOutput=False)
    WhhT = nc.declare_dram_parameter("WhhT", [KT, 128, G3], BF16, isOutput=False)
    bih = nc.declare_dram_parameter("bih", [128, MT], F32, isOutput=False)
    y_raw = nc.declare_dram_parameter("y_raw", [128, S * KT * BL], F32, isOutput=True)
    hx_raw = nc.declare_dram_parameter("hx_raw", [128, KT * BL], F32, isOutput=True)

    # Fully fused raw kernel: the input GEMM for chunk c+2 is interleaved
    # into recurrence chunk c (its matmuls fill the per-step PE bubble while
    # the gates tail computes h), writing gi directly into a 3-deep SBUF
    # ring -- no DRAM round-trip for gi at all.
    NG = MT * BL            # 192 gate columns per step
    NY = KT * BL            # 64
    NH = NY // 2            # 32
    TC = 32                 # steps per chunk (recurrence == gemm chunk)
    NCH = S // TC           # 32 chunks
    CW = TC * NY            # y columns per chunk (2048)
    NX = TC * BL            # gemm moving columns per chunk (256)
    GPS = 6                 # gemm matmuls inserted per step (192/TC)

    with contextlib.ExitStack() as ctx:
        def sb(name, shape, dt):
            return ctx.enter_context(nc.sbuf_tensor(name, shape, dt))

        whh = sb("whh", [128, KT * G3], BF16)
        wih = sb("wih", [128, ICT * G3], BF16)
        b_sb = sb("bsb", [128, MT], F32)
        gi3 = [sb(f"gi{i}", [128, TC, NG], F32) for i in range(3)]
        y_sb = [sb(f"ysb{i}", [128, CW], F32) for i in range(2)]
        xt = [sb(f"xt{i}", [128, ICT, NX], BF16) for i in range(2)]
        srz = [sb(f"srz{i}", [128, 128], F32) for i in range(2)]
        v0t = [sb(f"v0t{i}", [128, NH], F32) for i in range(2)]
        v1t = [sb(f"v1t{i}", [128, NH], F32) for i in range(2)]
        rzt = sb("rzt", [128, 128], F32)
        zh = sb("zht", [128, NY], F32)
        omz = sb("omzt", [128, NY], F32)
        u0t = sb("u0t", [128, NH], F32)
        u1t = sb("u1t", [128, NH], F32)
        nst0 = sb("nst0", [128, NH], F32)
        nst1 = sb("nst1", [128, NH], F32)
        hTf = sb("hTft", [128, NY], F32)
        hTb = sb("hTbt", [128, NY], BF16)
        # one full PSUM bank per tile; recurrence psums are single-buffered
        # (the hTb0 wait already serializes reuse across steps)
        ps_rza = ctx.enter_context(nc.psum_tensor("psrza", [128, 512], F32))
        ps_rzb = ctx.enter_context(nc.psum_tensor("psrzb", [128, 512], F32))
        ps_n0 = ctx.enter_context(nc.psum_tensor("psn0", [128, 512], F32))
        ps_n1 = ctx.enter_context(nc.psum_tensor("psn1", [128, 512], F32))
        gps = [
            ctx.enter_context(nc.psum_tensor(f"gps{i}", [128, 512], F32))
            for i in range(2)
        ]

        pe_s = ctx.enter_context(nc.semaphore("pe_s"))   # recurrence groups
        pg_s = ctx.enter_context(nc.semaphore("pg_s"))   # gemm m-groups
        ve_s = ctx.enter_context(nc.semaphore("ve_s"))   # recurrence DVE
        vg_s = ctx.enter_context(nc.semaphore("vg_s"))   # gemm scatters
        ac_s = ctx.enter_context(nc.semaphore("ac_s"))   # activations
        xd_s = ctx.enter_context(nc.semaphore("xd_s"))   # input DMAs
        y_s = ctx.enter_context(nc.semaphore("y_s"))     # y output DMAs

        # xd_s milestones
        XD_W = 16 * (ICT + KT + 2)            # weights + bias + h0 = 288
        def xd_xt(g):                          # xt(g) loaded
            return XD_W + 128 * (g + 1)

        def h_prev_ap(T):
            c, s = divmod(T, TC)
            if T == 0:
                return hTf[:]
            if s == 0:
                return y_sb[(c - 1) % 2][:, (TC - 1) * NY : TC * NY]
            return y_sb[c % 2][:, (s - 1) * NY : s * NY]

        # gemm matmul emission order for chunk g: (m, k) row-major
        def gemm_mm(tensor, g, idx):
            m, k = divmod(idx, KT)
            if idx == 0:
                tensor.wait_ge(xd_s, xd_xt(g))
            inst = tensor.matmul(
                gps[m % 2][:, 0:NX],
                wih[:, k * G3 + m * 128 : k * G3 + (m + 1) * 128],
                xt[g % 2][:, k, :],
                start=(k == 0),
                stop=(k == KT - 1),
                skip_group_check=True,
            )
            if k == 0:
                w = 24 * g + m - 1   # scatter of (g, m-2) done (psum WAR)
                if w >= 1:
                    inst._wait_ge(vg_s, w)
            if k == KT - 1:
                inst.then_inc(pg_s, 1)

        with nc.Block() as block:

            @block.sync
            def _(sync):
                for k in range(ICT):
                    sync.dma_start(
                        out=wih[:, k * G3 : (k + 1) * G3], in_=WihT[k]
                    ).then_inc(xd_s, 16)
                for k in range(KT):
                    sync.dma_start(
                        out=whh[:, k * G3 : (k + 1) * G3], in_=WhhT[k]
                    ).then_inc(xd_s, 16)
                sync.dma_start(out=b_sb[:], in_=bih[:]).then_inc(xd_s, 16)
                sync.dma_start(out=hTf[:], in_=h0T[:]).then_inc(xd_s, 16)
                for g in (0, 1):
                    for ic in range(ICT):
                        sync.dma_start(
                            out=xt[g % 2][:, ic, :],
                            in_=xT[ic][:, g * TC : (g + 1) * TC, :],
                        ).then_inc(xd_s, 16)
                # xt(2) overwrites xt(0): wait gemm(0) finished reading
                sync.wait_ge(pg_s, 24)
                for ic in range(ICT):
                    sync.dma_start(
                        out=xt[0][:, ic, :], in_=xT[ic][:, 2 * TC : 3 * TC, :]
                    ).then_inc(xd_s, 16)
                for c in range(NCH):
                    Tm = TC * c + (TC // 2 - 1)
                    Tl = TC * c + (TC - 1)
                    g = c + 3
                    if g < NCH:
                        # at chunk-c start: gemm(c+1) is already done, so
                        # xt(g)'s buffer (used by gemm(g-2)=gemm(c+1)) is free
                        sync.wait_ge(pg_s, 24 * (g - 1))
                        for ic in range(ICT):
                            sync.dma_start(
                                out=xt[g % 2][:, ic, :],
                                in_=xT[ic][:, g * TC : (g + 1) * TC, :],
                            ).then_inc(xd_s, 16)
                    sync.wait_ge(ve_s, 6 * Tm + 7)
                    sync.dma_start(
                        out=y_raw[:, c * CW : c * CW + CW // 2],
                        in_=y_sb[c % 2][:, : CW // 2],
                    ).then_inc(y_s, 16)
                    sync.wait_ge(ve_s, 6 * Tl + 7)
                    sync.dma_start(
                        out=y_raw[:, c * CW + CW // 2 : (c + 1) * CW],
                        in_=y_sb[c % 2][:, CW // 2 :],
                    ).then_inc(y_s, 16)
                sync.wait_ge(y_s, 32 * NCH)
                sync.dma_start(
                    out=hx_raw[:], in_=y_sb[(NCH - 1) % 2][:, (TC - 1) * NY :]
                ).then_inc(y_s, 16)
                sync.wait_ge(y_s, 32 * NCH + 16)

            @block.tensor
            def _(tensor):
                # prologue: gemm chunks 0 and 1 at full rate
                for g in (0, 1):
                    for idx in range(MT * KT):
                        gemm_mm(tensor, g, idx)

                def group(m_lo, m_hi, kr, pout, poff, wait_val):
                    first = True
                    last = None
                    for m in range(m_lo, m_hi):
                        for k in kr:
                            inst = tensor.matmul(
                                pout[:, (m - poff) * BL : (m - poff + 1) * BL],
                                whh[:, k * G3 + m * 128 : k * G3 + (m + 1) * 128],
                                hTb[:, k * BL : (k + 1) * BL],
                                start=(k == kr[0]),
                                stop=(k == kr[-1]),
                                skip_group_check=True,
                            )
                            if first and wait_val is not None:
                                inst._wait_ge(ve_s, wait_val)
                            first = False
                            last = inst
                    last.then_inc(pe_s, 1)

                for T in range(S):
                    c, s = divmod(T, TC)
                    wa = 1 if T == 0 else 6 * (T - 1) + 4
                    wb = 1 if T == 0 else 6 * (T - 1) + 6
                    group(0, 16, [0, 1, 2, 3], ps_rza, 0, wa)
                    group(0, 16, [4, 5, 6, 7], ps_rzb, 0, wb)
                    group(16, 20, [0, 1, 2, 3, 4, 5, 6, 7], ps_n0, 16, None)
                    group(20, 24, [0, 1, 2, 3, 4, 5, 6, 7], ps_n1, 20, None)
                    # fill the gates-tail bubble with gemm work for chunk c+2
                    g = c + 2
                    if g < NCH:
                        for idx in range(GPS * s, GPS * (s + 1)):
                            gemm_mm(tensor, g, idx)

            @block.vector
            def _(vector):
                vector.wait_ge(xd_s, XD_W)
                vector.tensor_copy(hTb[:], hTf[:]).then_inc(ve_s, 1)

                def scatter(g, m):
                    vector.tensor_scalar_add(
                        gi3[g % 3][:, :, m * BL : (m + 1) * BL],
                        gps[m % 2][:, 0:NX].rearrange("p (t b) -> p t b", b=BL),
                        b_sb[:, m : m + 1],
                    )._wait_ge(pg_s, 24 * g + m + 1).then_inc(vg_s, 1)

                for g in (0, 1):
                    for m in range(MT):
                        scatter(g, m)

                emitted = 0
                for T in range(S):
                    c, s = divmod(T, TC)
                    b = T % 2
                    gis = gi3[c % 3][:, s, :]
                    if s == 0 and c >= 2:
                        vector.wait_ge(y_s, 32 * (c - 1))
                    if s == 0:
                        vector.wait_ge(vg_s, 24 * (c + 1))
                    vector.tensor_add(
                        srz[b][:], ps_rza[:, 0:128], gis[:, 0:128]
                    )._wait_ge(pe_s, 4 * T + 1)
                    vector.tensor_add(srz[b][:], srz[b][:], ps_rzb[:, 0:128])._wait_ge(
                        pe_s, 4 * T + 2
                    ).then_inc(ve_s, 1)
                    vector.tensor_mul(zh[:], rzt[:, NY:128], h_prev_ap(T))._wait_ge(
                        ac_s, 3 * T + 1
                    )
                    vector.tensor_scalar(
                        omz[:], rzt[:, NY:128], -1.0, 1.0,
                        mybir.AluOpType.mult, mybir.AluOpType.add,
                    )
                    vector.tensor_mul(u0t[:], rzt[:, 0:NH], ps_n0[:, 0:NH])._wait_ge(
                        pe_s, 4 * T + 3
                    )
                    vector.tensor_add(
                        v0t[b][:], u0t[:], gis[:, 128 : 128 + NH]
                    ).then_inc(ve_s, 1)
                    # h10 precomputed so hTb0 (real dep: all-MMs WAR) fires fast
                    h10 = u0t
                    vector.tensor_mul(h10[:], nst0[:], omz[:, 0:NH])._wait_ge(
                        ac_s, 3 * T + 2
                    )
                    vector.tensor_add(hTb[:, 0:NH], h10[:], zh[:, 0:NH])._wait_ge(
                        pe_s, 4 * T + 4
                    ).then_inc(ve_s, 1)
                    vector.tensor_mul(u1t[:], rzt[:, NH:NY], ps_n1[:, 0:NH])
                    vector.tensor_add(
                        v1t[b][:], u1t[:], gis[:, 128 + NH : 128 + NY]
                    ).then_inc(ve_s, 1)
                    vector.tensor_mul(u1t[:], nst1[:], omz[:, NH:NY])._wait_ge(
                        ac_s, 3 * T + 3
                    )
                    vector.tensor_add(hTb[:, NH:NY], u1t[:], zh[:, NH:NY]).then_inc(
                        ve_s, 1
                    )
                    vector.tensor_add(
                        y_sb[c % 2][:, s * NY : s * NY + NH], h10[:], zh[:, 0:NH]
                    )
                    vector.tensor_add(
                        y_sb[c % 2][:, s * NY + NH : (s + 1) * NY],
                        u1t[:],
                        zh[:, NH:NY],
                    ).then_inc(ve_s, 1)
                    # interleave gemm scatters for chunk c+2
                    g = c + 2
                    if g < NCH:
                        want = (s + 1) * MT // TC
                        while emitted < want:
                            scatter(g, emitted)
                            emitted += 1
                        if s == TC - 1:
                            emitted = 0

            @block.scalar
            def _(scalar):
                for T in range(S):
                    b = T % 2
                    scalar.activation(rzt[:], srz[b][:], SIG)._wait_ge(
                        ve_s, 6 * T + 2
                    ).then_inc(ac_s, 1)
                    scalar.activation(nst0[:], v0t[b][:], TANH)._wait_ge(
                        ve_s, 6 * T + 3
                    ).then_inc(ac_s, 1)
                    scalar.activation(nst1[:], v1t[b][:], TANH)._wait_ge(
                        ve_s, 6 * T + 5
                    ).then_inc(ac_s, 1)

    nc.compile()
    return nc


_CACHE = {}


def _prep_inputs(x, h0, W_ih, W_hh, b_ih):
    bf = ml_dtypes.bfloat16
    x = np.ascontiguousarray(np.asarray(x, dtype=np.float32))
    h0 = np.asarray(h0, dtype=np.float32)
    W_ih = np.asarray(W_ih, dtype=np.float32)
    W_hh = np.asarray(W_hh, dtype=np.float32)
    b_ih = np.asarray(b_ih, dtype=np.float32)

    # shared weights
    wihT = np.ascontiguousarray(W_ih.T).reshape(ICT, 128, G3).astype(bf)
    whhT = np.ascontiguousarray(W_hh.T).reshape(KT, 128, G3).astype(bf)
    b_arr = np.ascontiguousarray(b_ih.reshape(MT, 128).T)

    in_maps = []
    for c in range(NC):
        x_c = x[c * BL : (c + 1) * BL]                      # [BL, S, I]
        xT_c = np.ascontiguousarray(x_c.transpose(2, 1, 0)) # [I, S, BL]
        xT_c = xT_c.reshape(ICT, 128, S, BL).astype(bf)
        h0_c = h0[c * BL : (c + 1) * BL]                    # [BL, H]
        h0T_c = np.ascontiguousarray(
            h0_c.reshape(BL, KT, 128).transpose(2, 1, 0).reshape(128, KT * BL)
        )
        in_maps.append(
            {"xT": xT_c, "h0T": h0T_c, "WihT": wihT, "WhhT": whhT, "bih": b_arr}
        )
    return in_maps


def _postprocess(results):
    y_full = np.empty((B, S, H), dtype=np.float32)
    hx_full = np.empty((B, H), dtype=np.float32)
    for c in range(NC):
        y_raw = results[c]["y_raw"]        # [128, S*KT*BL]
        hx_raw = results[c]["hx_raw"]      # [128, KT*BL]
        y = y_raw.reshape(128, S, KT, BL).transpose(3, 1, 2, 0).reshape(BL, S, H)
        y_full[c * BL : (c + 1) * BL] = y
        hx = hx_raw.reshape(128, KT, BL).transpose(2, 1, 0).reshape(BL, H)
        hx_full[c * BL : (c + 1) * BL] = hx
    return y_full, hx_full


def kernel(x, h0, W_ih, W_hh, b_ih):
    if "nc" not in _CACHE:
        _CACHE["nc"] = build()
    nc = _CACHE["nc"]
    in_maps = _prep_inputs(x, h0, W_ih, W_hh, b_ih)
    trace = bool(int(os.environ.get("GRU_TRACE", "0")))
    res = run_bass_kernel_spmd(nc, in_maps, list(range(NC)), trace=trace)
    _CACHE["last_result"] = res
    return _postprocess(res.results)


if __name__ == "__main__":
    rng = np.random.default_rng(0)
    sc = 1.0 / np.sqrt(H)
    inputs = {
        "x": rng.standard_normal((B, S, I), dtype=np.float32),
        "h0": np.zeros((B, H), dtype=np.float32),
        "W_ih": (rng.standard_normal((G3, I), dtype=np.float32) * sc),
        "W_hh": (rng.standard_normal((G3, H), dtype=np.float32) * sc),
        "b_ih": (rng.standard_normal(G3, dtype=np.float32) * sc),
    }
    y, hx = kernel(**inputs)
    print("ok", y.shape, hx.shape, float(np.abs(y).max()))
